# revision 13
# baseline (speedup 1.0000x reference)
"""Fused DDiT transformer block (causal) on 8 TRN2 NeuronCores.

Sharding: attention is head-parallel (2 heads/core, 16 total) with QKV
column-sliced per core; two AllToAlls (one per local head) re-shard from
head-split to token-split, and out-proj + MLP run token-parallel
(512 tokens/core). LayerNorm gains AND mean-centering are folded into the
following matmul weights on the host (W' = W - rowsum(W)/D), so only the
1/std factor is computed on device: each core computes LN1 sigma for its
own 512 tokens and an AllGather shares it; sigma folds into the RoPE
tables (q,k), a PSUM-eviction multiply (v), or a broadcast multiply (MLP).
Compute dtype bf16 (fp32 accumulation); the residual stream stays fp32.
"""
import sys

for _p in ("/opt/trn_rl_repo",):
    if _p not in sys.path:
        sys.path.append(_p)

import numpy as np
import ml_dtypes

import concourse.bass as bass
import concourse.tile as tile
import concourse.mybir as mybir
from concourse.bass_utils import run_bass_kernel_spmd
from concourse.masks import make_identity

bf16 = mybir.dt.bfloat16
f32 = mybir.dt.float32
AF = mybir.ActivationFunctionType
OP = mybir.AluOpType

N_CORES = 8
B, S, D = 2, 2048, 1024
T = B * S            # 4096 tokens total
NH, HD = 16, 64      # heads, head dim
HPC = NH // N_CORES  # 2 heads per core
TOK = T // N_CORES   # 512 tokens per core in the token-split phase
NT = T // 128        # 32 token tiles of 128
NCH = T // 512       # 8 chunks of 512 tokens
LN_EPS = 1e-5

# ---------------------------------------------------------------------------
# Sync legalizer: this walrus build accepts only ONE sync wait and ONE sync
# update per TPB instruction. Move extras onto same-engine NoOps (engines
# complete instructions in program order, so semantics are preserved).
# ---------------------------------------------------------------------------
_uid = [0]


def _legalize_sync(nc):
    for f in nc.m.functions:
        for bb in f.blocks:
            out = []
            changed = False
            for inst in bb.instructions:
                si = inst.sync_info
                if si is None:
                    out.append(inst)
                    continue
                waits = list(si.on_wait) if si.on_wait else []
                updates = list(si.on_update) if si.on_update else []
                if len(waits) <= 1 and len(updates) <= 1:
                    out.append(inst)
                    continue
                changed = True
                for w in waits[:-1]:
                    _uid[0] += 1
                    nop = mybir.InstNoOp(name=f"syncw-{_uid[0]}", ins=[], outs=[])
                    nop.engine = inst.engine
                    nop.sync_info = mybir.SyncInfo(on_wait=[w], on_update=[])
                    out.append(nop)
                inst.sync_info = mybir.SyncInfo(
                    on_wait=waits[-1:], on_update=updates[:1]
                )
                out.append(inst)
                for u in updates[1:]:
                    _uid[0] += 1
                    nop = mybir.InstNoOp(name=f"syncu-{_uid[0]}", ins=[], outs=[])
                    nop.engine = inst.engine
                    nop.sync_info = mybir.SyncInfo(on_wait=[], on_update=[u])
                    out.append(nop)
            if changed:
                bb.instructions = out
    return nc


# ---------------------------------------------------------------------------
# Kernel graph
# ---------------------------------------------------------------------------
def _build():
    nc = bass.Bass()

    # -- external inputs (per core)
    xT_blk = nc.dram_tensor("xT_blk", (NCH, 128, 8, 512), bf16, kind="ExternalInput")
    xT_own = nc.dram_tensor("xT_own", (D, TOK), f32, kind="ExternalInput")
    wqkv_blk = nc.dram_tensor("wqkv_blk", (3, 128, 8, 128), bf16, kind="ExternalInput")
    tab = nc.dram_tensor("tab", (2, 128, T), bf16, kind="ExternalInput")  # cos, sin(signed)
    wout_blk = nc.dram_tensor("wout_blk", (8, 128, 8, 128), bf16, kind="ExternalInput")
    w1_blk = nc.dram_tensor("w1_blk", (32, 128, 8, 128), bf16, kind="ExternalInput")
    b1_t = nc.dram_tensor("b1_t", (32, 128, 1), f32, kind="ExternalInput")
    w2_blk = nc.dram_tensor("w2_blk", (8, 128, 32, 128), bf16, kind="ExternalInput")
    b2_t = nc.dram_tensor("b2_t", (8, 128, 1), f32, kind="ExternalInput")
    out_d = nc.dram_tensor("out", (D, TOK), f32, kind="ExternalOutput")

    # -- internal DRAM
    ag_in = nc.dram_tensor("ag_in", (1, TOK), bf16, kind="Internal")
    ag_out = nc.dram_tensor("ag_out", (N_CORES, TOK), bf16, kind="Internal")
    cc_in = [nc.dram_tensor(f"cc_in{h}", (N_CORES, 64, TOK), bf16, kind="Internal")
             for h in range(2)]
    cc_out = [nc.dram_tensor(f"cc_out{h}", (N_CORES, 64, TOK), bf16, kind="Internal")
              for h in range(2)]

    with tile.TileContext(nc) as tc, \
         nc.allow_low_precision(reason="bf16 block compute"):
        with tc.tile_pool(name="const", bufs=1) as pconst, \
             tc.tile_pool(name="persist", bufs=1) as pper, \
             tc.tile_pool(name="stream", bufs=2) as pstream, \
             tc.tile_pool(name="big2", bufs=2) as pbig2, \
             tc.tile_pool(name="work", bufs=3) as pwork:
            # ---- residual stream + stats inputs first: tensor engine's
            # first work (LN1 stats) depends on these DMAs.
            xT_o = [pper.tile([128, 512], f32, name=f"xo_{kk}", tag=f"xo_{kk}")
                    for kk in range(8)]
            for kk in range(8):
                nc.sync.dma_start(out=xT_o[kk], in_=xT_own[kk * 128:(kk + 1) * 128, :])

            ident_bf = pconst.tile([128, 128], bf16)
            make_identity(nc, ident_bf)
            mask128 = pconst.tile([128, 128], bf16)
            nc.gpsimd.memset(mask128, 1.0)
            # causal: keep (1.0) where q_local - k_local = f - p >= 0
            nc.gpsimd.affine_select(
                out=mask128, in_=mask128, pattern=[[1, 128]],
                compare_op=OP.is_ge, fill=0.0, base=0, channel_multiplier=-1)
            ones_row = pconst.tile([1, 128], bf16)
            nc.vector.memset(ones_row, 1.0)
            ones_col = pconst.tile([128, 1], bf16)
            nc.vector.memset(ones_col, 1.0)
            eps1 = pconst.tile([1, 1], f32)
            nc.vector.memset(eps1, LN_EPS)

            # =============================================================
            # Phase A: LN1 sigma for OWN 512 tokens; AllGather across cores
            # =============================================================
            sd_row = pper.tile([1, T], bf16)
            with tc.tile_pool(name="psST", bufs=1, space="PSUM") as psST:
                ps_mu = psST.tile([1, 512], f32, tag="mu")
                ps_sq = psST.tile([1, 512], f32, tag="sq")
                for kk in range(8):
                    xb = pwork.tile([128, 512], bf16, tag="xb", bufs=2)
                    nc.vector.tensor_copy(out=xb, in_=xT_o[kk])
                    sqb = pwork.tile([128, 512], bf16, tag="sqb", bufs=2)
                    nc.vector.tensor_mul(out=sqb, in0=xb, in1=xb)
                    nc.tensor.matmul(ps_mu, ones_col, xb,
                                     start=(kk == 0), stop=(kk == 7))
                    nc.tensor.matmul(ps_sq, ones_col, sqb,
                                     start=(kk == 0), stop=(kk == 7))
                mu_r = pwork.tile([1, 512], f32, tag="mu_r", bufs=1)
                nc.vector.tensor_scalar_mul(out=mu_r, in0=ps_mu, scalar1=1.0 / D)
                mus_r = pwork.tile([1, 512], f32, tag="mus_r", bufs=1)
                nc.vector.tensor_mul(out=mus_r, in0=mu_r, in1=mu_r)
                var_r = pwork.tile([1, 512], f32, tag="var_r", bufs=1)
                nc.vector.scalar_tensor_tensor(
                    out=var_r, in0=ps_sq, scalar=1.0 / D, in1=mus_r,
                    op0=OP.mult, op1=OP.subtract)
                sd_own = pwork.tile([1, 512], bf16, tag="sd_own", bufs=1)
                nc.scalar.activation(out=sd_own, in_=var_r, func=AF.Sqrt,
                                     bias=eps1)
                nc.sync.dma_start(out=ag_in[0:1, :], in_=sd_own)
                nc.gpsimd.collective_compute(
                    "AllGather", OP.bypass, ins=[ag_in[:, :]], outs=[ag_out[:, :]],
                    replica_groups=[list(range(N_CORES))])
                for c in range(N_CORES):
                    nc.sync.dma_start(out=sd_row[0:1, c * 512:(c + 1) * 512],
                                      in_=ag_out[c:c + 1, :])

            # ---- weights / tables (prefetch while stats run); phase-B-only
            # tensors live in the attention-scoped pool so MLP gets the space
            attn_pool_cm = tc.tile_pool(name="attn", bufs=1)
            pattn = attn_pool_cm.__enter__()
            tabs = []
            for ti in range(2):
                raw = pattn.tile([128, T], bf16, name=f"tab{ti}", tag=f"tab{ti}")
                nc.sync.dma_start(out=raw, in_=tab[ti])
                tabs.append(raw)
            tab_c, tab_s = tabs
            wq_sb = {}
            for m in range(3):
                w = pattn.tile([128, 8, 128], bf16, name=f"wqkv_{m}", tag=f"wqkv_{m}")
                nc.sync.dma_start(out=w, in_=wqkv_blk[m])
                wq_sb[m] = w
            wo_sb = {}
            for m in range(8):
                w = pconst.tile([128, 8, 128], bf16, name=f"wout_{m}", tag=f"wout_{m}")
                nc.sync.dma_start(out=w[:, 0:4, :], in_=wout_blk[m, :, 0:4, :])
                nc.sync.dma_start(out=w[:, 4:8, :], in_=wout_blk[m, :, 4:8, :])
                wo_sb[m] = w
            b2_sb = {}
            for m in range(8):
                b = pconst.tile([128, 1], f32, name=f"b2_{m}", tag=f"b2_{m}")
                nc.sync.dma_start(out=b, in_=b2_t[m])
                b2_sb[m] = b

            # =============================================================
            # Phase B: QKV projection + RoPE + V transpose (pipelined)
            # =============================================================
            rstd_sb = pattn.tile([128, T], bf16)
            qT_sb = pattn.tile([128, T], bf16)
            kT_sb = pattn.tile([128, T], bf16)
            v_all = pattn.tile([128, NT, 130], bf16)

            with tc.tile_pool(name="psQKV", bufs=6, space="PSUM") as psQ, \
                 tc.tile_pool(name="psVT", bufs=1, space="PSUM") as psVT, \
                 tc.tile_pool(name="psB", bufs=1, space="PSUM") as psB:

                def mains(ch):
                    xrt = pbig2.tile([128, 8, 512], bf16, tag="xTr")
                    nc.sync.dma_start(out=xrt, in_=xT_blk[ch])
                    pss = []
                    for m in range(3):
                        ps = psQ.tile([128, 512], f32, tag="qkv")
                        for kk in range(8):
                            nc.tensor.matmul(ps, wq_sb[m][:, kk, :], xrt[:, kk, :],
                                             start=(kk == 0), stop=(kk == 7))
                        pss.append(ps)
                    return pss

                def bcastfold(ch):
                    sl = slice(ch * 512, (ch + 1) * 512)
                    ps_b = psB.tile([128, 512], f32, tag="bc")
                    nc.tensor.matmul(ps_b, ones_row[0:1, 0:128], sd_row[0:1, sl],
                                     start=True, stop=True)
                    nc.vector.reciprocal(out=rstd_sb[:, sl], in_=ps_b)
                    nc.vector.tensor_mul(out=tab_c[:, sl], in0=tab_c[:, sl],
                                         in1=rstd_sb[:, sl])
                    nc.vector.tensor_mul(out=tab_s[:, sl], in0=tab_s[:, sl],
                                         in1=rstd_sb[:, sl])

                def evict(ch, pss):
                    sl = slice(ch * 512, (ch + 1) * 512)
                    for m in range(2):  # q, k: rope
                        ps = pss[m]
                        dst = qT_sb if m == 0 else kT_sb
                        tc_t = pwork.tile([128, 512], bf16, tag="ropec", bufs=2)
                        nc.vector.tensor_copy(out=tc_t, in_=ps)
                        tsw = pwork.tile([128, 512], bf16, tag="ropesw", bufs=2)
                        for h in range(2):
                            for a2 in range(2):
                                nc.vector.tensor_copy(
                                    out=tsw[h * 64 + a2 * 32:h * 64 + a2 * 32 + 32, :],
                                    in_=tc_t[h * 64 + (1 - a2) * 32:h * 64 + (1 - a2) * 32 + 32, :])
                        t1 = pwork.tile([128, 512], bf16, tag="ropet1", bufs=2)
                        nc.vector.tensor_mul(out=t1, in0=ps, in1=tab_c[:, sl])
                        t2 = pwork.tile([128, 512], bf16, tag="ropet2", bufs=2)
                        nc.vector.tensor_mul(out=t2, in0=tsw, in1=tab_s[:, sl])
                        nc.vector.tensor_add(out=dst[:, sl], in0=t1, in1=t2)
                    # v: scale by rstd, transpose to [t, e] tiles
                    vt = pwork.tile([128, 512], bf16, tag="vtmp")
                    nc.vector.tensor_mul(out=vt, in0=pss[2], in1=rstd_sb[:, sl])
                    for j in range(4):
                        g = ch * 4 + j
                        pst = psVT.tile([128, 128], bf16, tag="vtr")
                        nc.tensor.transpose(out=pst, in_=vt[:, j * 128:(j + 1) * 128],
                                            identity=ident_bf)
                        nc.vector.tensor_copy(out=v_all[:, g, 0:64], in_=pst[:, 0:64])
                        nc.vector.tensor_copy(out=v_all[:, g, 65:129], in_=pst[:, 64:128])
                        nc.vector.memset(v_all[:, g, 64:65], 1.0)
                        nc.vector.memset(v_all[:, g, 129:130], 1.0)

                prev = mains(0)
                bcastfold(0)
                for ch in range(1, NCH):
                    cur = mains(ch)
                    evict(ch - 1, prev)
                    bcastfold(ch)
                    prev = cur
                evict(NCH - 1, prev)

            # =============================================================
            # Phase C: causal attention per (head, batch), transposed layout
            # h-outer so each head-half AllToAll can start at half-time
            # =============================================================
            with tc.tile_pool(name="psSC", bufs=4, space="PSUM") as psSC, \
                 tc.tile_pool(name="psO", bufs=2, space="PSUM") as psO, \
                 tc.tile_pool(name="psBC", bufs=2, space="PSUM") as psBC:
                for h in range(2):
                    hsl = slice(h * 64, (h + 1) * 64)
                    for b in range(2):
                        for qc in range(4):
                            qsl = slice(b * 2048 + qc * 512, b * 2048 + (qc + 1) * 512)
                            nkt = 4 * (qc + 1)
                            ps_o = psO.tile([65, 512], f32, tag="o")
                            p_prev = None
                            for kt in range(nkt):
                                ps_s = psSC.tile([128, 512], f32, tag="sc")
                                ksl = slice(b * 2048 + kt * 128, b * 2048 + (kt + 1) * 128)
                                nc.tensor.matmul(ps_s, kT_sb[hsl, ksl], qT_sb[hsl, qsl],
                                                 start=True, stop=True)
                                p_t = pwork.tile([128, 512], bf16, tag="p", bufs=6)
                                nc.scalar.activation(out=p_t, in_=ps_s, func=AF.Exp)
                                if kt >= 4 * qc:  # diagonal block: causal mask
                                    off = kt * 128 - qc * 512
                                    if off > 0:
                                        nc.vector.memset(p_t[:, 0:off], 0.0)
                                    nc.vector.tensor_mul(
                                        out=p_t[:, off:off + 128],
                                        in0=p_t[:, off:off + 128], in1=mask128)
                                if p_prev is not None:
                                    g, pp, first = p_prev
                                    nc.tensor.matmul(
                                        ps_o, v_all[:, g, h * 65:(h + 1) * 65], pp,
                                        start=first, stop=False)
                                p_prev = (b * 16 + kt, p_t, kt == 0)
                            g, pp, first = p_prev
                            nc.tensor.matmul(ps_o, v_all[:, g, h * 65:(h + 1) * 65], pp,
                                             start=first, stop=True)
                            # normalize: broadcast sums, wide reciprocal, mul
                            sums = pwork.tile([1, 512], bf16, tag="sums", bufs=2)
                            nc.vector.tensor_copy(out=sums, in_=ps_o[64:65, :])
                            ps_b = psBC.tile([64, 512], f32, tag="ob")
                            nc.tensor.matmul(ps_b, ones_row[0:1, 0:64], sums,
                                             start=True, stop=True)
                            rec = pwork.tile([64, 512], bf16, tag="rec", bufs=2)
                            nc.vector.reciprocal(out=rec, in_=ps_b)
                            o_t = pwork.tile([64, 512], bf16, tag="o_t")
                            nc.vector.tensor_mul(out=o_t, in0=ps_o[0:64, :], in1=rec)
                            j = b * 4 + qc
                            nc.sync.dma_start(out=cc_in[h][j], in_=o_t)
                    nc.gpsimd.collective_compute(
                        "AllToAll", OP.bypass, ins=[cc_in[h][:, :, :]],
                        outs=[cc_out[h][:, :, :]],
                        replica_groups=[list(range(N_CORES))])

            attn_pool_cm.__exit__(None, None, None)
            mlp_pool_cm = tc.tile_pool(name="mlp", bufs=1)
            pmlp = mlp_pool_cm.__enter__()

            # =============================================================
            # Phase D: out-proj (token-split), residual, LN2 stats
            # =============================================================
            o_own = [pmlp.tile([128, 512], bf16, name=f"oo_{kk}", tag=f"oo_{kk}")
                     for kk in range(8)]
            for kk in range(8):
                nc.sync.dma_start(out=o_own[kk][0:64, :], in_=cc_out[0][kk])
                nc.sync.dma_start(out=o_own[kk][64:128, :], in_=cc_out[1][kk])

            xa = [pmlp.tile([128, 512], f32, name=f"xa_{m}", tag=f"xa_{m}") for m in range(8)]
            xab = [pmlp.tile([128, 512], bf16, name=f"xab_{m}", tag=f"xab_{m}") for m in range(8)]
            with tc.tile_pool(name="psOP", bufs=2, space="PSUM") as psOP, \
                 tc.tile_pool(name="psMU", bufs=1, space="PSUM") as psMU, \
                 tc.tile_pool(name="psSQ", bufs=1, space="PSUM") as psSQ, \
                 tc.tile_pool(name="psRB", bufs=1, space="PSUM") as psRB:
                ps_mu = psMU.tile([1, 512], f32)
                ps_sq = psSQ.tile([1, 512], f32)
                for m in range(8):
                    ps = psOP.tile([128, 512], f32, tag="op")
                    for kk in range(8):
                        nc.tensor.matmul(ps, wo_sb[m][:, kk, :], o_own[kk],
                                         start=(kk == 0), stop=(kk == 7))
                    nc.vector.tensor_add(out=xa[m], in0=ps, in1=xT_o[m])
                    nc.vector.tensor_copy(out=xab[m], in_=xa[m])
                    sq = pwork.tile([128, 512], bf16, tag="sq", bufs=2)
                    nc.vector.tensor_mul(out=sq, in0=xab[m], in1=xab[m])
                    nc.tensor.matmul(ps_mu, ones_col, xab[m],
                                     start=(m == 0), stop=(m == 7))
                    nc.tensor.matmul(ps_sq, ones_col, sq,
                                     start=(m == 0), stop=(m == 7))
                    # fold the mlp2 bias into the residual now (stats above
                    # already consumed the pre-bias value via xab/sq)
                    nc.scalar.activation(out=xa[m], in_=xa[m], func=AF.Identity,
                                         bias=b2_sb[m])

                # LN2 row stats: mu = sum/1024, var = sqsum/1024 - mu^2
                mu2 = pwork.tile([1, 512], f32, tag="mu2", bufs=1)
                nc.vector.tensor_scalar_mul(out=mu2, in0=ps_mu, scalar1=1.0 / D)
                mus_r = pwork.tile([1, 512], f32, tag="mus2_r", bufs=1)
                nc.vector.tensor_mul(out=mus_r, in0=mu2, in1=mu2)
                var_r = pwork.tile([1, 512], f32, tag="var2_r", bufs=1)
                nc.vector.scalar_tensor_tensor(
                    out=var_r, in0=ps_sq, scalar=1.0 / D, in1=mus_r,
                    op0=OP.mult, op1=OP.subtract)
                sd2 = pwork.tile([1, 512], bf16, tag="sd2_r", bufs=1)
                nc.scalar.activation(out=sd2, in_=var_r, func=AF.Sqrt, bias=eps1)
                ps_rb = psRB.tile([128, 512], f32)
                nc.tensor.matmul(ps_rb, ones_row[0:1, 0:128], sd2, start=True, stop=True)
                rstd2_sb = pmlp.tile([128, 512], bf16)
                nc.vector.reciprocal(out=rstd2_sb, in_=ps_rb)

            # =============================================================
            # Phase E: MLP (token-split, full weights)
            # =============================================================
            u_g = [pmlp.tile([128, 512], bf16, name=f"ug_{m}", tag=f"ug_{m}") for m in range(32)]
            with tc.tile_pool(name="psU", bufs=3, space="PSUM") as psU, \
                 tc.tile_pool(name="psDn", bufs=2, space="PSUM") as psDn:
                for m in range(32):
                    ps = psU.tile([128, 512], f32, tag="u")
                    w = pstream.tile([128, 8, 128], bf16, tag="w1_st", bufs=4)
                    nc.sync.dma_start(out=w[:, 0:4, :], in_=w1_blk[m, :, 0:4, :])
                    nc.sync.dma_start(out=w[:, 4:8, :], in_=w1_blk[m, :, 4:8, :])
                    for kk in range(8):
                        nc.tensor.matmul(ps, w[:, kk, :], xab[kk],
                                         start=(kk == 0), stop=(kk == 7))
                    upre = pwork.tile([128, 512], bf16, tag="upre", bufs=2)
                    nc.vector.tensor_mul(out=upre, in0=ps, in1=rstd2_sb)
                    b1 = pwork.tile([128, 1], f32, tag="b1_st")
                    nc.sync.dma_start(out=b1, in_=b1_t[m])
                    nc.scalar.activation(out=u_g[m], in_=upre,
                                         func=AF.Gelu_apprx_tanh, bias=b1)
                for m in range(8):
                    ps = psDn.tile([128, 512], f32, tag="dn")
                    w = pstream.tile([128, 32, 128], bf16, tag="w2_st", bufs=2)
                    for q4 in range(4):
                        nc.sync.dma_start(out=w[:, q4 * 8:(q4 + 1) * 8, :],
                                          in_=w2_blk[m, :, q4 * 8:(q4 + 1) * 8, :])
                    for kk in range(32):
                        nc.tensor.matmul(ps, w[:, kk, :], u_g[kk],
                                         start=(kk == 0), stop=(kk == 31))
                    ot = pwork.tile([128, 512], f32, tag="ot", bufs=2)
                    nc.vector.tensor_add(out=ot, in0=ps, in1=xa[m])
                    nc.sync.dma_start(out=out_d[m * 128:(m + 1) * 128, :], in_=ot)
            mlp_pool_cm.__exit__(None, None, None)

    _legalize_sync(nc)
    return nc


# ---------------------------------------------------------------------------
# Host-side prep + execution
# ---------------------------------------------------------------------------
_NC_CACHE = {}


def _get_nc():
    if "nc" not in _NC_CACHE:
        _NC_CACHE["nc"] = _build()
    return _NC_CACHE["nc"]


def _bf(a):
    return np.ascontiguousarray(a).astype(ml_dtypes.bfloat16)


def _f32(a):
    return np.ascontiguousarray(a, dtype=np.float32)


def _mean_fold(w):
    # W' = W - rowsum(W)/D : folds LN mean-centering into the matmul
    return w - w.sum(1, keepdims=True) / w.shape[1]


def _prep_inputs(x, rot_cos, rot_sin, ln1_w, w_qkv, w_out, ln2_w, w_mlp1,
                 b_mlp1, w_mlp2, b_mlp2):
    x = np.asarray(x, np.float32)
    X = x.reshape(T, D)

    xT = X.T  # (D, T)
    # (ch, p, kk, t): partition row p holds all kk-blocks contiguously
    xT_blk = _bf(xT.reshape(8, 128, NCH, 512).transpose(2, 1, 0, 3))

    # rope tables: (128 rows = 2 heads x [first32|last32]) x T tokens
    cos = np.asarray(rot_cos, np.float32)[0, :, 0, 0, :HD // 2]  # (S, 32)
    sin = np.asarray(rot_sin, np.float32)[0, :, 0, 0, :HD // 2]
    cT = np.concatenate([cos, cos], 1).T          # (64, S)
    sT = np.concatenate([-sin, sin], 1).T         # (64, S) sign-folded
    cT = np.tile(cT, (2, B))                      # (128, T)
    sT = np.tile(sT, (2, B))
    tab = _bf(np.stack([cT, sT]))

    wqkv_eff = np.asarray(w_qkv, np.float32) * np.asarray(ln1_w, np.float32)[None, :]
    w1_eff = np.asarray(w_mlp1, np.float32) * np.asarray(ln2_w, np.float32)[None, :]
    w1_eff = _mean_fold(w1_eff)
    w_out_f = np.asarray(w_out, np.float32)
    w2_f = np.asarray(w_mlp2, np.float32)

    woutT = w_out_f.T  # (d_in=head dims, e)
    wout_blk = _bf(woutT.reshape(8, 128, 8, 128).transpose(2, 1, 0, 3))  # [m, p, kk, e]
    w1T = w1_eff.T     # (D, 4D)
    w1_blk = _bf(w1T.reshape(8, 128, 32, 128).transpose(2, 1, 0, 3))
    w2T = w2_f.T       # (4D, D)
    w2_blk = _bf(w2T.reshape(32, 128, 8, 128).transpose(2, 1, 0, 3))
    b1_arr = _f32(np.asarray(b_mlp1, np.float32).reshape(32, 128, 1))
    b2_arr = _f32(np.asarray(b_mlp2, np.float32).reshape(8, 128, 1))

    in_maps = []
    for c in range(N_CORES):
        w_sl = np.concatenate(
            [wqkv_eff[0 * D + 2 * c * HD: 0 * D + 2 * (c + 1) * HD] * 0.125,
             wqkv_eff[1 * D + 2 * c * HD: 1 * D + 2 * (c + 1) * HD],
             wqkv_eff[2 * D + 2 * c * HD: 2 * D + 2 * (c + 1) * HD]], 0)  # (384, D)
        w_sl = _mean_fold(w_sl)
        wT_sl = w_sl.T  # (D, 384) -> [m, p, kk, e]
        wqkv_b = _bf(wT_sl.reshape(8, 128, 3, 128).transpose(2, 1, 0, 3))
        in_maps.append({
            "xT_blk": xT_blk,
            "xT_own": _f32(xT[:, c * TOK:(c + 1) * TOK]),
            "wqkv_blk": wqkv_b,
            "tab": tab,
            "wout_blk": wout_blk,
            "w1_blk": w1_blk,
            "b1_t": b1_arr,
            "w2_blk": w2_blk,
            "b2_t": b2_arr,
        })
    return in_maps


def _assemble(results):
    outT = np.concatenate([results[c]["out"] for c in range(N_CORES)], axis=1)
    return np.ascontiguousarray(outT.T.astype(np.float32)).reshape(B, S, D)


def run_spmd(in_maps, **kwargs):
    nc = _get_nc()
    return run_bass_kernel_spmd(nc, in_maps, core_ids=list(range(N_CORES)), **kwargs)


def kernel(x, rot_cos, rot_sin, ln1_w, w_qkv, w_out, ln2_w, w_mlp1, b_mlp1,
           w_mlp2, b_mlp2):
    in_maps = _prep_inputs(x, rot_cos, rot_sin, ln1_w, w_qkv, w_out, ln2_w,
                           w_mlp1, b_mlp1, w_mlp2, b_mlp2)
    res = run_spmd(in_maps)
    return _assemble(res.results)


# revision 29
# speedup vs baseline: 1.1781x; 1.1781x over previous
"""Fused DDiT transformer block (causal) on 8 TRN2 NeuronCores.

Sharding: attention is head-parallel (2 heads/core, 16 total) with QKV
column-sliced per core; two AllToAlls (one per local head) re-shard from
head-split to token-split, and out-proj + MLP run token-parallel
(512 tokens/core). LayerNorm gains AND mean-centering are folded into the
following matmul weights on the host (W' = W - rowsum(W)/D), so only the
1/std factor is computed on device: each core computes LN1 sigma for its
own 512 tokens and an AllGather shares it; sigma folds into the RoPE
tables (q,k), a PSUM-eviction multiply (v), or a broadcast multiply (MLP).
Compute dtype bf16 (fp32 accumulation); the residual stream stays fp32.
"""
import sys

for _p in ("/opt/trn_rl_repo",):
    if _p not in sys.path:
        sys.path.append(_p)

import numpy as np
import ml_dtypes

import concourse.bass as bass
import concourse.tile as tile
import concourse.mybir as mybir
from concourse.bass_utils import run_bass_kernel_spmd
from concourse.masks import make_identity

bf16 = mybir.dt.bfloat16
f32 = mybir.dt.float32
AF = mybir.ActivationFunctionType
OP = mybir.AluOpType

N_CORES = 8
B, S, D = 2, 2048, 1024
T = B * S            # 4096 tokens total
NH, HD = 16, 64      # heads, head dim
HPC = NH // N_CORES  # 2 heads per core
TOK = T // N_CORES   # 512 tokens per core in the token-split phase
NT = T // 128        # 32 token tiles of 128
NCH = T // 512       # 8 chunks of 512 tokens
LN_EPS = 1e-5

# ---------------------------------------------------------------------------
# Sync legalizer: this walrus build accepts only ONE sync wait and ONE sync
# update per TPB instruction. Move extras onto same-engine NoOps (engines
# complete instructions in program order, so semantics are preserved).
# ---------------------------------------------------------------------------
_uid = [0]


def _legalize_sync(nc):
    for f in nc.m.functions:
        for bb in f.blocks:
            out = []
            changed = False
            for inst in bb.instructions:
                si = inst.sync_info
                if si is None:
                    out.append(inst)
                    continue
                waits = list(si.on_wait) if si.on_wait else []
                updates = list(si.on_update) if si.on_update else []
                if len(waits) <= 1 and len(updates) <= 1:
                    out.append(inst)
                    continue
                changed = True
                for w in waits[:-1]:
                    _uid[0] += 1
                    nop = mybir.InstNoOp(name=f"syncw-{_uid[0]}", ins=[], outs=[])
                    nop.engine = inst.engine
                    nop.sync_info = mybir.SyncInfo(on_wait=[w], on_update=[])
                    out.append(nop)
                inst.sync_info = mybir.SyncInfo(
                    on_wait=waits[-1:], on_update=updates[:1]
                )
                out.append(inst)
                for u in updates[1:]:
                    _uid[0] += 1
                    nop = mybir.InstNoOp(name=f"syncu-{_uid[0]}", ins=[], outs=[])
                    nop.engine = inst.engine
                    nop.sync_info = mybir.SyncInfo(on_wait=[], on_update=[u])
                    out.append(nop)
            if changed:
                bb.instructions = out
    return nc


# ---------------------------------------------------------------------------
# Kernel graph
# ---------------------------------------------------------------------------
def _build():
    nc = bass.Bass()

    # -- external inputs (per core)
    xT_blk = nc.dram_tensor("xT_blk", (NCH, 128, 8, 512), bf16, kind="ExternalInput")
    xT_own = nc.dram_tensor("xT_own", (D, TOK), f32, kind="ExternalInput")
    wqkv_blk = nc.dram_tensor("wqkv_blk", (3, 128, 8, 128), bf16, kind="ExternalInput")
    tab = nc.dram_tensor("tab", (2, 128, T), bf16, kind="ExternalInput")  # cos, sin(signed)
    wout_blk = nc.dram_tensor("wout_blk", (8, 128, 8, 128), bf16, kind="ExternalInput")
    w1_blk = nc.dram_tensor("w1_blk", (32, 128, 8, 128), bf16, kind="ExternalInput")
    b1_t = nc.dram_tensor("b1_t", (32, 128, 1), f32, kind="ExternalInput")
    w2_blk = nc.dram_tensor("w2_blk", (8, 128, 32, 128), bf16, kind="ExternalInput")
    b2_t = nc.dram_tensor("b2_t", (8, 128, 1), f32, kind="ExternalInput")
    out_d = nc.dram_tensor("out", (D, TOK), f32, kind="ExternalOutput")

    # -- internal DRAM
    ag_in = nc.dram_tensor("ag_in", (1, TOK), bf16, kind="Internal")
    ag_out = nc.dram_tensor("ag_out", (N_CORES, TOK), bf16, kind="Internal")
    cc_in = [nc.dram_tensor(f"cc_in{h}", (N_CORES, 64, TOK), bf16, kind="Internal")
             for h in range(2)]
    cc_out = [nc.dram_tensor(f"cc_out{h}", (N_CORES, 64, TOK), bf16, kind="Internal")
              for h in range(2)]

    with tile.TileContext(nc) as tc, \
         nc.allow_low_precision(reason="bf16 block compute"):
        with tc.tile_pool(name="const", bufs=1) as pconst, \
             tc.tile_pool(name="persist", bufs=1) as pper, \
             tc.tile_pool(name="stream", bufs=2) as pstream, \
             tc.tile_pool(name="big2", bufs=2) as pbig2, \
             tc.tile_pool(name="work", bufs=3) as pwork:
            # ---- residual stream + stats inputs first: tensor engine's
            # first work (LN1 stats) depends on these DMAs.
            xT_o = [pper.tile([128, 512], f32, name=f"xo_{kk}", tag=f"xo_{kk}")
                    for kk in range(8)]
            for kk in range(8):
                nc.sync.dma_start(out=xT_o[kk], in_=xT_own[kk * 128:(kk + 1) * 128, :])

            ident_bf = pconst.tile([128, 128], bf16)
            make_identity(nc, ident_bf)
            mask128 = pconst.tile([128, 128], bf16)
            nc.gpsimd.memset(mask128, 1.0)
            # causal: keep (1.0) where q_local - k_local = f - p >= 0
            nc.gpsimd.affine_select(
                out=mask128, in_=mask128, pattern=[[1, 128]],
                compare_op=OP.is_ge, fill=0.0, base=0, channel_multiplier=-1)
            ones_row = pconst.tile([1, 128], bf16)
            nc.vector.memset(ones_row, 1.0)
            ones_col = pconst.tile([128, 1], bf16)
            nc.vector.memset(ones_col, 1.0)
            eps1 = pconst.tile([1, 1], f32)
            nc.vector.memset(eps1, LN_EPS)

            # =============================================================
            # Phase A: LN1 sigma for OWN 512 tokens; AllGather across cores
            # =============================================================
            rstd_row = pper.tile([1, T], bf16)
            with tc.tile_pool(name="psST", bufs=1, space="PSUM") as psST:
                ps_mu = psST.tile([1, 512], f32, tag="mu")
                ps_sq = psST.tile([1, 512], f32, tag="sq")
                for kk in range(8):
                    xb = pwork.tile([128, 512], bf16, tag="xb", bufs=2)
                    nc.scalar.activation(out=xb, in_=xT_o[kk], func=AF.Copy)
                    sqb = pwork.tile([128, 512], bf16, tag="sqb", bufs=2)
                    nc.vector.tensor_mul(out=sqb, in0=xb, in1=xb)
                    nc.tensor.matmul(ps_mu, ones_col, xb,
                                     start=(kk == 0), stop=(kk == 7))
                    nc.tensor.matmul(ps_sq, ones_col, sqb,
                                     start=(kk == 0), stop=(kk == 7))
                mu_r = pwork.tile([1, 512], f32, tag="mu_r", bufs=1)
                nc.vector.tensor_scalar_mul(out=mu_r, in0=ps_mu, scalar1=1.0 / D)
                mus_r = pwork.tile([1, 512], f32, tag="mus_r", bufs=1)
                nc.vector.tensor_mul(out=mus_r, in0=mu_r, in1=mu_r)
                var_r = pwork.tile([1, 512], f32, tag="var_r", bufs=1)
                nc.vector.scalar_tensor_tensor(
                    out=var_r, in0=ps_sq, scalar=1.0 / D, in1=mus_r,
                    op0=OP.mult, op1=OP.subtract)
                sd_own = pwork.tile([1, 512], f32, tag="sd_own", bufs=1)
                nc.scalar.activation(out=sd_own, in_=var_r, func=AF.Sqrt,
                                     bias=eps1)
                rstd_own = pwork.tile([1, 512], bf16, tag="rstd_own", bufs=1)
                nc.vector.reciprocal(out=rstd_own, in_=sd_own)
                nc.sync.dma_start(out=ag_in[0:1, :], in_=rstd_own)
                nc.gpsimd.collective_compute(
                    "AllGather", OP.bypass, ins=[ag_in[:, :]], outs=[ag_out[:, :]],
                    replica_groups=[list(range(N_CORES))])
                for c in range(N_CORES):
                    nc.sync.dma_start(out=rstd_row[0:1, c * 512:(c + 1) * 512],
                                      in_=ag_out[c:c + 1, :])

            # ---- weights / tables (prefetch while stats run); phase-B-only
            # tensors live in the attention-scoped pool so MLP gets the space
            attn_pool_cm = tc.tile_pool(name="attn", bufs=1)
            pattn = attn_pool_cm.__enter__()
            tabs = []
            for ti in range(2):
                raw = pattn.tile([128, T], bf16, name=f"tab{ti}", tag=f"tab{ti}")
                nc.sync.dma_start(out=raw, in_=tab[ti])
                tabs.append(raw)
            tab_c, tab_s = tabs
            wq_sb = {}
            for m in range(3):
                w = pattn.tile([128, 8, 128], bf16, name=f"wqkv_{m}", tag=f"wqkv_{m}")
                nc.sync.dma_start(out=w, in_=wqkv_blk[m])
                wq_sb[m] = w
            wo_sb = {}
            for m in range(8):
                w = pconst.tile([128, 8, 128], bf16, name=f"wout_{m}", tag=f"wout_{m}")
                nc.sync.dma_start(out=w[:, 0:4, :], in_=wout_blk[m, :, 0:4, :])
                nc.sync.dma_start(out=w[:, 4:8, :], in_=wout_blk[m, :, 4:8, :])
                wo_sb[m] = w
            b2_sb = {}
            for m in range(8):
                b = pconst.tile([128, 1], f32, name=f"b2_{m}", tag=f"b2_{m}")
                nc.sync.dma_start(out=b, in_=b2_t[m])
                b2_sb[m] = b

            # =============================================================
            # Phase B: QKV projection + RoPE + V transpose (pipelined)
            # =============================================================
            rstd_sb = pattn.tile([128, T], bf16)
            qT_sb = pattn.tile([128, T], bf16)
            kT_sb = pattn.tile([128, T], bf16)
            v_all = pattn.tile([128, NT, 130], bf16)

            with tc.tile_pool(name="psQKV", bufs=6, space="PSUM") as psQ, \
                 tc.tile_pool(name="psVT", bufs=1, space="PSUM") as psVT, \
                 tc.tile_pool(name="psB", bufs=1, space="PSUM") as psB:

                def mains(ch):
                    xrt = pbig2.tile([128, 8, 512], bf16, tag="xTr")
                    nc.sync.dma_start(out=xrt, in_=xT_blk[ch])
                    pss = []
                    for m in range(3):
                        ps = psQ.tile([128, 512], f32, tag="qkv")
                        for kk in range(8):
                            nc.tensor.matmul(ps, wq_sb[m][:, kk, :], xrt[:, kk, :],
                                             start=(kk == 0), stop=(kk == 7))
                        pss.append(ps)
                    return pss

                def bcastfold(ch):
                    sl = slice(ch * 512, (ch + 1) * 512)
                    ps_b = psB.tile([128, 512], f32, tag="bc")
                    nc.tensor.matmul(ps_b, ones_row[0:1, 0:128], rstd_row[0:1, sl],
                                     start=True, stop=True)
                    nc.scalar.activation(out=rstd_sb[:, sl], in_=ps_b, func=AF.Copy)
                    nc.vector.tensor_mul(out=tab_c[:, sl], in0=tab_c[:, sl],
                                         in1=rstd_sb[:, sl])
                    nc.vector.tensor_mul(out=tab_s[:, sl], in0=tab_s[:, sl],
                                         in1=rstd_sb[:, sl])

                def evict(ch, pss):
                    sl = slice(ch * 512, (ch + 1) * 512)
                    for m in range(2):  # q, k: rope
                        ps = pss[m]
                        dst = qT_sb if m == 0 else kT_sb
                        tc_t = pwork.tile([128, 512], bf16, tag="ropec", bufs=2)
                        nc.scalar.activation(out=tc_t, in_=ps, func=AF.Copy)
                        tsw = pwork.tile([128, 512], bf16, tag="ropesw", bufs=2)
                        for h in range(2):
                            for a2 in range(2):
                                nc.vector.tensor_copy(
                                    out=tsw[h * 64 + a2 * 32:h * 64 + a2 * 32 + 32, :],
                                    in_=tc_t[h * 64 + (1 - a2) * 32:h * 64 + (1 - a2) * 32 + 32, :])
                        t1 = pwork.tile([128, 512], bf16, tag="ropet1", bufs=2)
                        nc.vector.tensor_mul(out=t1, in0=ps, in1=tab_c[:, sl])
                        t2 = pwork.tile([128, 512], bf16, tag="ropet2", bufs=2)
                        nc.vector.tensor_mul(out=t2, in0=tsw, in1=tab_s[:, sl])
                        nc.vector.tensor_add(out=dst[:, sl], in0=t1, in1=t2)
                    # v: scale by rstd, transpose to [t, e] tiles
                    vt = pwork.tile([128, 512], bf16, tag="vtmp")
                    nc.vector.tensor_mul(out=vt, in0=pss[2], in1=rstd_sb[:, sl])
                    for j in range(4):
                        g = ch * 4 + j
                        pst = psVT.tile([128, 128], bf16, tag="vtr")
                        nc.tensor.transpose(out=pst, in_=vt[:, j * 128:(j + 1) * 128],
                                            identity=ident_bf)
                        nc.vector.tensor_copy(out=v_all[:, g, 0:64], in_=pst[:, 0:64])
                        nc.vector.tensor_copy(out=v_all[:, g, 65:129], in_=pst[:, 64:128])
                        nc.vector.memset(v_all[:, g, 64:65], 1.0)
                        nc.vector.memset(v_all[:, g, 129:130], 1.0)

                prev = mains(0)
                cur = mains(1)
                bcastfold(0)
                for ch in range(2, NCH):
                    nxt = mains(ch)
                    evict(ch - 2, prev)
                    bcastfold(ch - 1)
                    prev, cur = cur, nxt
                evict(NCH - 2, prev)
                bcastfold(NCH - 1)
                evict(NCH - 1, cur)

            # =============================================================
            # Phase C: causal attention per (head, batch), transposed layout
            # h-outer so each head-half AllToAll can start at half-time
            # =============================================================
            with tc.tile_pool(name="psSC", bufs=4, space="PSUM") as psSC, \
                 tc.tile_pool(name="psO", bufs=2, space="PSUM") as psO, \
                 tc.tile_pool(name="psBC", bufs=2, space="PSUM") as psBC:
                for h in range(2):
                    hsl = slice(h * 64, (h + 1) * 64)
                    for b in range(2):
                        for qc in range(4):
                            qsl = slice(b * 2048 + qc * 512, b * 2048 + (qc + 1) * 512)
                            nkt = 4 * (qc + 1)
                            ps_o = psO.tile([65, 512], f32, tag="o")
                            p_prev = None
                            for kt in range(nkt):
                                ps_s = psSC.tile([128, 512], f32, tag="sc")
                                ksl = slice(b * 2048 + kt * 128, b * 2048 + (kt + 1) * 128)
                                nc.tensor.matmul(ps_s, kT_sb[hsl, ksl], qT_sb[hsl, qsl],
                                                 start=True, stop=True)
                                p_t = pwork.tile([128, 512], bf16, tag="p", bufs=6)
                                nc.scalar.activation(out=p_t, in_=ps_s, func=AF.Exp)
                                if kt >= 4 * qc:  # diagonal block: causal mask
                                    off = kt * 128 - qc * 512
                                    if off > 0:
                                        nc.vector.memset(p_t[:, 0:off], 0.0)
                                    nc.vector.tensor_mul(
                                        out=p_t[:, off:off + 128],
                                        in0=p_t[:, off:off + 128], in1=mask128)
                                if p_prev is not None:
                                    g, pp, first = p_prev
                                    nc.tensor.matmul(
                                        ps_o, v_all[:, g, h * 65:(h + 1) * 65], pp,
                                        start=first, stop=False)
                                p_prev = (b * 16 + kt, p_t, kt == 0)
                            g, pp, first = p_prev
                            nc.tensor.matmul(ps_o, v_all[:, g, h * 65:(h + 1) * 65], pp,
                                             start=first, stop=True)
                            # evict PSUM fast (frees the bank), then the slow
                            # DVE reciprocal runs parallel to the exp stream
                            o_u = pwork.tile([64, 512], bf16, tag="o_u", bufs=3)
                            nc.vector.tensor_copy(out=o_u, in_=ps_o[0:64, :])
                            sums = pwork.tile([1, 512], f32, tag="sums", bufs=2)
                            nc.vector.tensor_copy(out=sums, in_=ps_o[64:65, :])
                            rrow = pwork.tile([1, 512], bf16, tag="rrow", bufs=2)
                            nc.vector.reciprocal(out=rrow, in_=sums)
                            ps_b = psBC.tile([64, 512], f32, tag="ob")
                            nc.tensor.matmul(ps_b, ones_row[0:1, 0:64], rrow,
                                             start=True, stop=True)
                            o_t = pwork.tile([64, 512], bf16, tag="o_t")
                            nc.vector.tensor_mul(out=o_t, in0=o_u, in1=ps_b)
                            j = b * 4 + qc
                            nc.sync.dma_start(out=cc_in[h][j], in_=o_t)
                    nc.gpsimd.collective_compute(
                        "AllToAll", OP.bypass, ins=[cc_in[h][:, :, :]],
                        outs=[cc_out[h][:, :, :]],
                        replica_groups=[list(range(N_CORES))])

            attn_pool_cm.__exit__(None, None, None)
            mlp_pool_cm = tc.tile_pool(name="mlp", bufs=1)
            pmlp = mlp_pool_cm.__enter__()

            # =============================================================
            # Phase D: out-proj (token-split), residual, LN2 stats
            # =============================================================
            o_own = [pmlp.tile([128, 512], bf16, name=f"oo_{kk}", tag=f"oo_{kk}")
                     for kk in range(8)]
            for kk in range(8):
                nc.sync.dma_start(out=o_own[kk][0:64, :], in_=cc_out[0][kk])
                nc.sync.dma_start(out=o_own[kk][64:128, :], in_=cc_out[1][kk])

            xa = [pmlp.tile([128, 512], f32, name=f"xa_{m}", tag=f"xa_{m}") for m in range(8)]
            xab = [pmlp.tile([128, 512], bf16, name=f"xab_{m}", tag=f"xab_{m}") for m in range(8)]
            with tc.tile_pool(name="psOP", bufs=2, space="PSUM") as psOP, \
                 tc.tile_pool(name="psMU", bufs=1, space="PSUM") as psMU, \
                 tc.tile_pool(name="psSQ", bufs=1, space="PSUM") as psSQ, \
                 tc.tile_pool(name="psRB", bufs=1, space="PSUM") as psRB:
                ps_mu = psMU.tile([1, 512], f32)
                ps_sq = psSQ.tile([1, 512], f32)
                for m in range(8):
                    ps = psOP.tile([128, 512], f32, tag="op")
                    for kk in range(8):
                        nc.tensor.matmul(ps, wo_sb[m][:, kk, :], o_own[kk],
                                         start=(kk == 0), stop=(kk == 7))
                    nc.vector.tensor_add(out=xa[m], in0=ps, in1=xT_o[m])
                    nc.vector.tensor_copy(out=xab[m], in_=xa[m])
                    sq = pwork.tile([128, 512], bf16, tag="sq", bufs=2)
                    nc.vector.tensor_mul(out=sq, in0=xab[m], in1=xab[m])
                    nc.tensor.matmul(ps_mu, ones_col, xab[m],
                                     start=(m == 0), stop=(m == 7))
                    nc.tensor.matmul(ps_sq, ones_col, sq,
                                     start=(m == 0), stop=(m == 7))
                    # fold the mlp2 bias into the residual now (stats above
                    # already consumed the pre-bias value via xab/sq)
                    nc.scalar.activation(out=xa[m], in_=xa[m], func=AF.Identity,
                                         bias=b2_sb[m])

                # LN2 row stats: mu = sum/1024, var = sqsum/1024 - mu^2
                mu2 = pwork.tile([1, 512], f32, tag="mu2", bufs=1)
                nc.vector.tensor_scalar_mul(out=mu2, in0=ps_mu, scalar1=1.0 / D)
                mus_r = pwork.tile([1, 512], f32, tag="mus2_r", bufs=1)
                nc.vector.tensor_mul(out=mus_r, in0=mu2, in1=mu2)
                var_r = pwork.tile([1, 512], f32, tag="var2_r", bufs=1)
                nc.vector.scalar_tensor_tensor(
                    out=var_r, in0=ps_sq, scalar=1.0 / D, in1=mus_r,
                    op0=OP.mult, op1=OP.subtract)
                sd2 = pwork.tile([1, 512], f32, tag="sd2_r", bufs=1)
                nc.scalar.activation(out=sd2, in_=var_r, func=AF.Sqrt, bias=eps1)
                rstd2 = pwork.tile([1, 512], bf16, tag="rstd2_r", bufs=1)
                nc.vector.reciprocal(out=rstd2, in_=sd2)
                ps_rb = psRB.tile([128, 512], f32)
                nc.tensor.matmul(ps_rb, ones_row[0:1, 0:128], rstd2, start=True, stop=True)
                rstd2_sb = pmlp.tile([128, 512], bf16)
                nc.vector.tensor_copy(out=rstd2_sb, in_=ps_rb)

            # =============================================================
            # Phase E: MLP (token-split, full weights)
            # =============================================================
            u_g = [pmlp.tile([128, 512], bf16, name=f"ug_{m}", tag=f"ug_{m}") for m in range(32)]
            with tc.tile_pool(name="psU", bufs=3, space="PSUM") as psU, \
                 tc.tile_pool(name="psDn", bufs=2, space="PSUM") as psDn:
                for m in range(32):
                    ps = psU.tile([128, 512], f32, tag="u")
                    w = pstream.tile([128, 8, 128], bf16, tag="w1_st", bufs=4)
                    nc.sync.dma_start(out=w[:, 0:4, :], in_=w1_blk[m, :, 0:4, :])
                    nc.sync.dma_start(out=w[:, 4:8, :], in_=w1_blk[m, :, 4:8, :])
                    for kk in range(8):
                        nc.tensor.matmul(ps, w[:, kk, :], xab[kk],
                                         start=(kk == 0), stop=(kk == 7))
                    upre = pwork.tile([128, 512], bf16, tag="upre", bufs=2)
                    nc.vector.tensor_mul(out=upre, in0=ps, in1=rstd2_sb)
                    b1 = pwork.tile([128, 1], f32, tag="b1_st")
                    nc.sync.dma_start(out=b1, in_=b1_t[m])
                    nc.scalar.activation(out=u_g[m], in_=upre,
                                         func=AF.Gelu_apprx_tanh, bias=b1)
                for m in range(8):
                    ps = psDn.tile([128, 512], f32, tag="dn")
                    w = pstream.tile([128, 32, 128], bf16, tag="w2_st", bufs=2)
                    for q4 in range(4):
                        nc.sync.dma_start(out=w[:, q4 * 8:(q4 + 1) * 8, :],
                                          in_=w2_blk[m, :, q4 * 8:(q4 + 1) * 8, :])
                    for kk in range(32):
                        nc.tensor.matmul(ps, w[:, kk, :], u_g[kk],
                                         start=(kk == 0), stop=(kk == 31))
                    ot = pwork.tile([128, 512], f32, tag="ot", bufs=2)
                    nc.vector.tensor_add(out=ot, in0=ps, in1=xa[m])
                    nc.sync.dma_start(out=out_d[m * 128:(m + 1) * 128, :], in_=ot)
            mlp_pool_cm.__exit__(None, None, None)

    _legalize_sync(nc)
    return nc


# ---------------------------------------------------------------------------
# Host-side prep + execution
# ---------------------------------------------------------------------------
_NC_CACHE = {}


def _get_nc():
    if "nc" not in _NC_CACHE:
        _NC_CACHE["nc"] = _build()
    return _NC_CACHE["nc"]


def _bf(a):
    return np.ascontiguousarray(a).astype(ml_dtypes.bfloat16)


def _f32(a):
    return np.ascontiguousarray(a, dtype=np.float32)


def _mean_fold(w):
    # W' = W - rowsum(W)/D : folds LN mean-centering into the matmul
    return w - w.sum(1, keepdims=True) / w.shape[1]


def _prep_inputs(x, rot_cos, rot_sin, ln1_w, w_qkv, w_out, ln2_w, w_mlp1,
                 b_mlp1, w_mlp2, b_mlp2):
    x = np.asarray(x, np.float32)
    X = x.reshape(T, D)

    xT = X.T  # (D, T)
    # (ch, p, kk, t): partition row p holds all kk-blocks contiguously
    xT_blk = _bf(xT.reshape(8, 128, NCH, 512).transpose(2, 1, 0, 3))

    # rope tables: (128 rows = 2 heads x [first32|last32]) x T tokens
    cos = np.asarray(rot_cos, np.float32)[0, :, 0, 0, :HD // 2]  # (S, 32)
    sin = np.asarray(rot_sin, np.float32)[0, :, 0, 0, :HD // 2]
    cT = np.concatenate([cos, cos], 1).T          # (64, S)
    sT = np.concatenate([-sin, sin], 1).T         # (64, S) sign-folded
    cT = np.tile(cT, (2, B))                      # (128, T)
    sT = np.tile(sT, (2, B))
    tab = _bf(np.stack([cT, sT]))

    wqkv_eff = np.asarray(w_qkv, np.float32) * np.asarray(ln1_w, np.float32)[None, :]
    w1_eff = np.asarray(w_mlp1, np.float32) * np.asarray(ln2_w, np.float32)[None, :]
    w1_eff = _mean_fold(w1_eff)
    w_out_f = np.asarray(w_out, np.float32)
    w2_f = np.asarray(w_mlp2, np.float32)

    woutT = w_out_f.T  # (d_in=head dims, e)
    wout_blk = _bf(woutT.reshape(8, 128, 8, 128).transpose(2, 1, 0, 3))  # [m, p, kk, e]
    w1T = w1_eff.T     # (D, 4D)
    w1_blk = _bf(w1T.reshape(8, 128, 32, 128).transpose(2, 1, 0, 3))
    w2T = w2_f.T       # (4D, D)
    w2_blk = _bf(w2T.reshape(32, 128, 8, 128).transpose(2, 1, 0, 3))
    b1_arr = _f32(np.asarray(b_mlp1, np.float32).reshape(32, 128, 1))
    b2_arr = _f32(np.asarray(b_mlp2, np.float32).reshape(8, 128, 1))

    in_maps = []
    for c in range(N_CORES):
        w_sl = np.concatenate(
            [wqkv_eff[0 * D + 2 * c * HD: 0 * D + 2 * (c + 1) * HD] * 0.125,
             wqkv_eff[1 * D + 2 * c * HD: 1 * D + 2 * (c + 1) * HD],
             wqkv_eff[2 * D + 2 * c * HD: 2 * D + 2 * (c + 1) * HD]], 0)  # (384, D)
        w_sl = _mean_fold(w_sl)
        wT_sl = w_sl.T  # (D, 384) -> [m, p, kk, e]
        wqkv_b = _bf(wT_sl.reshape(8, 128, 3, 128).transpose(2, 1, 0, 3))
        in_maps.append({
            "xT_blk": xT_blk,
            "xT_own": _f32(xT[:, c * TOK:(c + 1) * TOK]),
            "wqkv_blk": wqkv_b,
            "tab": tab,
            "wout_blk": wout_blk,
            "w1_blk": w1_blk,
            "b1_t": b1_arr,
            "w2_blk": w2_blk,
            "b2_t": b2_arr,
        })
    return in_maps


def _assemble(results):
    outT = np.concatenate([results[c]["out"] for c in range(N_CORES)], axis=1)
    return np.ascontiguousarray(outT.T.astype(np.float32)).reshape(B, S, D)


def run_spmd(in_maps, **kwargs):
    nc = _get_nc()
    return run_bass_kernel_spmd(nc, in_maps, core_ids=list(range(N_CORES)), **kwargs)


def kernel(x, rot_cos, rot_sin, ln1_w, w_qkv, w_out, ln2_w, w_mlp1, b_mlp1,
           w_mlp2, b_mlp2):
    in_maps = _prep_inputs(x, rot_cos, rot_sin, ln1_w, w_qkv, w_out, ln2_w,
                           w_mlp1, b_mlp1, w_mlp2, b_mlp2)
    res = run_spmd(in_maps)
    return _assemble(res.results)


# revision 36
# speedup vs baseline: 1.1809x; 1.0024x over previous
"""Fused DDiT transformer block (causal) on 8 TRN2 NeuronCores.

Sharding: attention is head-parallel (2 heads/core, 16 total) with QKV
column-sliced per core; two AllToAlls (one per local head) re-shard from
head-split to token-split, and out-proj + MLP run token-parallel
(512 tokens/core). LayerNorm gains AND mean-centering are folded into the
following matmul weights on the host (W' = W - rowsum(W)/D), so only the
1/std factor is computed on device: each core computes LN1 sigma for its
own 512 tokens and an AllGather shares it; sigma folds into the RoPE
tables (q,k), a PSUM-eviction multiply (v), or a broadcast multiply (MLP).
Compute dtype bf16 (fp32 accumulation); the residual stream stays fp32.
"""
import sys

for _p in ("/opt/trn_rl_repo",):
    if _p not in sys.path:
        sys.path.append(_p)

import numpy as np
import ml_dtypes

import concourse.bass as bass
import concourse.tile as tile
import concourse.mybir as mybir
from concourse.bass_utils import run_bass_kernel_spmd
from concourse.masks import make_identity

bf16 = mybir.dt.bfloat16
f32 = mybir.dt.float32
AF = mybir.ActivationFunctionType
OP = mybir.AluOpType

N_CORES = 8
B, S, D = 2, 2048, 1024
T = B * S            # 4096 tokens total
NH, HD = 16, 64      # heads, head dim
HPC = NH // N_CORES  # 2 heads per core
TOK = T // N_CORES   # 512 tokens per core in the token-split phase
NT = T // 128        # 32 token tiles of 128
NCH = T // 512       # 8 chunks of 512 tokens
LN_EPS = 1e-5

# ---------------------------------------------------------------------------
# Sync legalizer: this walrus build accepts only ONE sync wait and ONE sync
# update per TPB instruction. Move extras onto same-engine NoOps (engines
# complete instructions in program order, so semantics are preserved).
# ---------------------------------------------------------------------------
_uid = [0]


def _legalize_sync(nc):
    for f in nc.m.functions:
        for bb in f.blocks:
            out = []
            changed = False
            for inst in bb.instructions:
                si = inst.sync_info
                if si is None:
                    out.append(inst)
                    continue
                waits = list(si.on_wait) if si.on_wait else []
                updates = list(si.on_update) if si.on_update else []
                if len(waits) <= 1 and len(updates) <= 1:
                    out.append(inst)
                    continue
                changed = True
                for w in waits[:-1]:
                    _uid[0] += 1
                    nop = mybir.InstNoOp(name=f"syncw-{_uid[0]}", ins=[], outs=[])
                    nop.engine = inst.engine
                    nop.sync_info = mybir.SyncInfo(on_wait=[w], on_update=[])
                    out.append(nop)
                inst.sync_info = mybir.SyncInfo(
                    on_wait=waits[-1:], on_update=updates[:1]
                )
                out.append(inst)
                for u in updates[1:]:
                    _uid[0] += 1
                    nop = mybir.InstNoOp(name=f"syncu-{_uid[0]}", ins=[], outs=[])
                    nop.engine = inst.engine
                    nop.sync_info = mybir.SyncInfo(on_wait=[], on_update=[u])
                    out.append(nop)
            if changed:
                bb.instructions = out
    return nc


# ---------------------------------------------------------------------------
# Kernel graph
# ---------------------------------------------------------------------------
def _build():
    nc = bass.Bass()

    # -- external inputs (per core)
    xT_blk = nc.dram_tensor("xT_blk", (NCH, 128, 8, 512), bf16, kind="ExternalInput")
    xT_own = nc.dram_tensor("xT_own", (D, TOK), f32, kind="ExternalInput")
    wqkv_blk = nc.dram_tensor("wqkv_blk", (3, 128, 8, 128), bf16, kind="ExternalInput")
    tab = nc.dram_tensor("tab", (2, 128, T), bf16, kind="ExternalInput")  # cos, sin(signed)
    wout_blk = nc.dram_tensor("wout_blk", (8, 128, 8, 128), bf16, kind="ExternalInput")
    w1_blk = nc.dram_tensor("w1_blk", (32, 128, 8, 128), bf16, kind="ExternalInput")
    b1_t = nc.dram_tensor("b1_t", (32, 128, 1), f32, kind="ExternalInput")
    w2_blk = nc.dram_tensor("w2_blk", (8, 128, 32, 128), bf16, kind="ExternalInput")
    b2_t = nc.dram_tensor("b2_t", (8, 128, 1), f32, kind="ExternalInput")
    out_d = nc.dram_tensor("out", (D, TOK), f32, kind="ExternalOutput")

    # -- internal DRAM
    ag_in = nc.dram_tensor("ag_in", (1, TOK), bf16, kind="Internal")
    ag_out = nc.dram_tensor("ag_out", (N_CORES, TOK), bf16, kind="Internal")
    cc_in = [nc.dram_tensor(f"cc_in{h}", (N_CORES, 64, TOK), bf16, kind="Internal")
             for h in range(2)]
    cc_out = [nc.dram_tensor(f"cc_out{h}", (N_CORES, 64, TOK), bf16, kind="Internal")
              for h in range(2)]

    with tile.TileContext(nc) as tc, \
         nc.allow_low_precision(reason="bf16 block compute"):
        with tc.tile_pool(name="const", bufs=1) as pconst, \
             tc.tile_pool(name="persist", bufs=1) as pper, \
             tc.tile_pool(name="stream", bufs=2) as pstream, \
             tc.tile_pool(name="big2", bufs=2) as pbig2, \
             tc.tile_pool(name="work", bufs=3) as pwork:
            # ---- residual stream + stats inputs first: tensor engine's
            # first work (LN1 stats) depends on these DMAs.
            xT_o = [pper.tile([128, 512], f32, name=f"xo_{kk}", tag=f"xo_{kk}")
                    for kk in range(8)]
            for kk in range(8):
                nc.sync.dma_start(out=xT_o[kk], in_=xT_own[kk * 128:(kk + 1) * 128, :])

            ident_bf = pconst.tile([128, 128], bf16)
            make_identity(nc, ident_bf)
            ones_row = pconst.tile([1, 128], bf16)
            nc.vector.memset(ones_row, 1.0)
            ones_col = pconst.tile([128, 1], bf16)
            nc.vector.memset(ones_col, 1.0)
            eps1 = pconst.tile([1, 1], f32)
            nc.vector.memset(eps1, LN_EPS)

            # =============================================================
            # Phase A: LN1 sigma for OWN 512 tokens; AllGather across cores
            # =============================================================
            rstd_row = pper.tile([1, T], bf16)
            with tc.tile_pool(name="psST", bufs=1, space="PSUM") as psST:
                ps_mu = psST.tile([1, 512], f32, tag="mu")
                ps_sq = psST.tile([1, 512], f32, tag="sq")
                for kk in range(8):
                    xb = pwork.tile([128, 512], bf16, tag="xb", bufs=2)
                    nc.scalar.activation(out=xb, in_=xT_o[kk], func=AF.Copy)
                    sqb = pwork.tile([128, 512], bf16, tag="sqb", bufs=2)
                    nc.vector.tensor_mul(out=sqb, in0=xb, in1=xb)
                    nc.tensor.matmul(ps_mu, ones_col, xb,
                                     start=(kk == 0), stop=(kk == 7))
                    nc.tensor.matmul(ps_sq, ones_col, sqb,
                                     start=(kk == 0), stop=(kk == 7))
                mu_r = pwork.tile([1, 512], f32, tag="mu_r", bufs=1)
                nc.vector.tensor_scalar_mul(out=mu_r, in0=ps_mu, scalar1=1.0 / D)
                mus_r = pwork.tile([1, 512], f32, tag="mus_r", bufs=1)
                nc.vector.tensor_mul(out=mus_r, in0=mu_r, in1=mu_r)
                var_r = pwork.tile([1, 512], f32, tag="var_r", bufs=1)
                nc.vector.scalar_tensor_tensor(
                    out=var_r, in0=ps_sq, scalar=1.0 / D, in1=mus_r,
                    op0=OP.mult, op1=OP.subtract)
                sd_own = pwork.tile([1, 512], f32, tag="sd_own", bufs=1)
                nc.scalar.activation(out=sd_own, in_=var_r, func=AF.Sqrt,
                                     bias=eps1)
                rstd_own = pwork.tile([1, 512], bf16, tag="rstd_own", bufs=1)
                nc.vector.reciprocal(out=rstd_own, in_=sd_own)
                nc.sync.dma_start(out=ag_in[0:1, :], in_=rstd_own)
                nc.gpsimd.collective_compute(
                    "AllGather", OP.bypass, ins=[ag_in[:, :]], outs=[ag_out[:, :]],
                    replica_groups=[list(range(N_CORES))])
                for c in range(N_CORES):
                    nc.sync.dma_start(out=rstd_row[0:1, c * 512:(c + 1) * 512],
                                      in_=ag_out[c:c + 1, :])

            # ---- weights / tables (prefetch while stats run); phase-B-only
            # tensors live in the attention-scoped pool so MLP gets the space
            attn_pool_cm = tc.tile_pool(name="attn", bufs=1)
            pattn = attn_pool_cm.__enter__()
            # full-width causal masks for the 4 diagonal offsets: for a k-tile
            # at columns [off, off+128) keep where q - k = f - p - off >= 0
            maskfull = []
            for oi in range(4):
                mf = pattn.tile([128, 512], bf16, name=f"mask{oi}", tag=f"mask{oi}")
                nc.gpsimd.memset(mf, 1.0)
                nc.gpsimd.affine_select(
                    out=mf, in_=mf, pattern=[[1, 512]],
                    compare_op=OP.is_ge, fill=0.0, base=-oi * 128,
                    channel_multiplier=-1)
                maskfull.append(mf)
            tabs = [pattn.tile([128, T], bf16, name=f"tab{ti}", tag=f"tab{ti}")
                    for ti in range(2)]
            tab_c, tab_s = tabs
            wq_sb = {}
            for m in range(3):
                w = pattn.tile([128, 8, 128], bf16, name=f"wqkv_{m}", tag=f"wqkv_{m}")
                nc.sync.dma_start(out=w, in_=wqkv_blk[m])
                wq_sb[m] = w
            # wout/b2 tiles allocated now, DMA'd during attention (phase C)
            wo_sb = {m: pconst.tile([128, 8, 128], bf16, name=f"wout_{m}",
                                    tag=f"wout_{m}") for m in range(8)}
            b2_sb = {m: pconst.tile([128, 1], f32, name=f"b2_{m}", tag=f"b2_{m}")
                     for m in range(8)}

            # =============================================================
            # Phase B: QKV projection + RoPE + V transpose (pipelined)
            # =============================================================
            rstd_sb = pattn.tile([128, T], bf16)
            qT_sb = pattn.tile([128, T], bf16)
            kT_sb = pattn.tile([128, T], bf16)
            v_all = pattn.tile([128, NT, 130], bf16)

            with tc.tile_pool(name="psQKV", bufs=6, space="PSUM") as psQ, \
                 tc.tile_pool(name="psVT", bufs=1, space="PSUM") as psVT, \
                 tc.tile_pool(name="psB", bufs=1, space="PSUM") as psB:

                def mains(ch):
                    xrt = pbig2.tile([128, 8, 512], bf16, tag="xTr")
                    nc.sync.dma_start(out=xrt, in_=xT_blk[ch])
                    pss = []
                    for m in range(3):
                        ps = psQ.tile([128, 512], f32, tag="qkv")
                        for kk in range(8):
                            nc.tensor.matmul(ps, wq_sb[m][:, kk, :], xrt[:, kk, :],
                                             start=(kk == 0), stop=(kk == 7))
                        pss.append(ps)
                    return pss

                def bcastfold(ch):
                    sl = slice(ch * 512, (ch + 1) * 512)
                    ps_b = psB.tile([128, 512], f32, tag="bc")
                    nc.tensor.matmul(ps_b, ones_row[0:1, 0:128], rstd_row[0:1, sl],
                                     start=True, stop=True)
                    nc.scalar.activation(out=rstd_sb[:, sl], in_=ps_b, func=AF.Copy)
                    nc.vector.tensor_mul(out=tab_c[:, sl], in0=tab_c[:, sl],
                                         in1=rstd_sb[:, sl])
                    nc.vector.tensor_mul(out=tab_s[:, sl], in0=tab_s[:, sl],
                                         in1=rstd_sb[:, sl])

                def evict(ch, pss):
                    sl = slice(ch * 512, (ch + 1) * 512)
                    for m in range(2):  # q, k: rope
                        ps = pss[m]
                        dst = qT_sb if m == 0 else kT_sb
                        tc_t = pwork.tile([128, 512], bf16, tag="ropec", bufs=2)
                        nc.scalar.activation(out=tc_t, in_=ps, func=AF.Copy)
                        tsw = pwork.tile([128, 512], bf16, tag="ropesw", bufs=2)
                        for h in range(2):
                            for a2 in range(2):
                                nc.vector.tensor_copy(
                                    out=tsw[h * 64 + a2 * 32:h * 64 + a2 * 32 + 32, :],
                                    in_=tc_t[h * 64 + (1 - a2) * 32:h * 64 + (1 - a2) * 32 + 32, :])
                        t1 = pwork.tile([128, 512], bf16, tag="ropet1", bufs=2)
                        nc.vector.tensor_mul(out=t1, in0=ps, in1=tab_c[:, sl])
                        t2 = pwork.tile([128, 512], bf16, tag="ropet2", bufs=2)
                        nc.vector.tensor_mul(out=t2, in0=tsw, in1=tab_s[:, sl])
                        nc.vector.tensor_add(out=dst[:, sl], in0=t1, in1=t2)
                    # v: scale by rstd, transpose to [t, e] tiles
                    vt = pwork.tile([128, 512], bf16, tag="vtmp")
                    nc.vector.tensor_mul(out=vt, in0=pss[2], in1=rstd_sb[:, sl])
                    for j in range(4):
                        g = ch * 4 + j
                        pst = psVT.tile([128, 128], bf16, tag="vtr")
                        nc.tensor.transpose(out=pst, in_=vt[:, j * 128:(j + 1) * 128],
                                            identity=ident_bf)
                        nc.vector.tensor_copy(out=v_all[:, g, 0:64], in_=pst[:, 0:64])
                        nc.vector.tensor_copy(out=v_all[:, g, 65:129], in_=pst[:, 64:128])
                        nc.vector.memset(v_all[:, g, 64:65], 1.0)
                        nc.vector.memset(v_all[:, g, 129:130], 1.0)

                prev = mains(0)
                cur = mains(1)
                # rope tables arrive behind the first two x chunks
                for ti in range(2):
                    nc.sync.dma_start(out=tabs[ti], in_=tab[ti])
                bcastfold(0)
                for ch in range(2, NCH):
                    nxt = mains(ch)
                    evict(ch - 2, prev)
                    bcastfold(ch - 1)
                    prev, cur = cur, nxt
                evict(NCH - 2, prev)
                bcastfold(NCH - 1)
                evict(NCH - 1, cur)

            # =============================================================
            # Phase C: causal attention per (head, batch), transposed layout
            # h-outer so each head-half AllToAll can start at half-time
            # =============================================================
            # out-proj weights/bias stream in while attention computes
            for m in range(8):
                nc.sync.dma_start(out=wo_sb[m][:, 0:4, :], in_=wout_blk[m, :, 0:4, :])
                nc.sync.dma_start(out=wo_sb[m][:, 4:8, :], in_=wout_blk[m, :, 4:8, :])
                nc.sync.dma_start(out=b2_sb[m], in_=b2_t[m])

            with tc.tile_pool(name="psSC", bufs=4, space="PSUM") as psSC, \
                 tc.tile_pool(name="psO", bufs=2, space="PSUM") as psO, \
                 tc.tile_pool(name="psBC", bufs=2, space="PSUM") as psBC:
                for h in range(2):
                    hsl = slice(h * 64, (h + 1) * 64)
                    for b in range(2):
                        for qc in range(4):
                            qsl = slice(b * 2048 + qc * 512, b * 2048 + (qc + 1) * 512)
                            nkt = 4 * (qc + 1)
                            ps_o = psO.tile([65, 512], f32, tag="o")
                            p_prev = None
                            for kt in range(nkt):
                                ps_s = psSC.tile([128, 512], f32, tag="sc")
                                ksl = slice(b * 2048 + kt * 128, b * 2048 + (kt + 1) * 128)
                                nc.tensor.matmul(ps_s, kT_sb[hsl, ksl], qT_sb[hsl, qsl],
                                                 start=True, stop=True)
                                p_t = pwork.tile([128, 512], bf16, tag="p", bufs=6)
                                nc.scalar.activation(out=p_t, in_=ps_s, func=AF.Exp)
                                if kt >= 4 * qc:  # diagonal block: causal mask
                                    # on gpsimd: keeps the PV path off the DVE
                                    # queue (which holds the slow reciprocals)
                                    nc.gpsimd.tensor_mul(
                                        out=p_t, in0=p_t, in1=maskfull[kt - 4 * qc])
                                if p_prev is not None:
                                    g, pp, first = p_prev
                                    nc.tensor.matmul(
                                        ps_o, v_all[:, g, h * 65:(h + 1) * 65], pp,
                                        start=first, stop=False)
                                p_prev = (b * 16 + kt, p_t, kt == 0)
                            g, pp, first = p_prev
                            nc.tensor.matmul(ps_o, v_all[:, g, h * 65:(h + 1) * 65], pp,
                                             start=first, stop=True)
                            # evict PSUM fast (frees the bank), then the slow
                            # DVE reciprocal runs parallel to the exp stream
                            o_u = pwork.tile([64, 512], bf16, tag="o_u", bufs=3)
                            nc.vector.tensor_copy(out=o_u, in_=ps_o[0:64, :])
                            sums = pwork.tile([1, 512], f32, tag="sums", bufs=2)
                            nc.vector.tensor_copy(out=sums, in_=ps_o[64:65, :])
                            rrow = pwork.tile([1, 512], bf16, tag="rrow", bufs=2)
                            nc.vector.reciprocal(out=rrow, in_=sums)
                            ps_b = psBC.tile([64, 512], f32, tag="ob")
                            nc.tensor.matmul(ps_b, ones_row[0:1, 0:64], rrow,
                                             start=True, stop=True)
                            o_t = pwork.tile([64, 512], bf16, tag="o_t")
                            nc.vector.tensor_mul(out=o_t, in0=o_u, in1=ps_b)
                            j = b * 4 + qc
                            nc.sync.dma_start(out=cc_in[h][j], in_=o_t)
                    nc.gpsimd.collective_compute(
                        "AllToAll", OP.bypass, ins=[cc_in[h][:, :, :]],
                        outs=[cc_out[h][:, :, :]],
                        replica_groups=[list(range(N_CORES))])

            attn_pool_cm.__exit__(None, None, None)
            mlp_pool_cm = tc.tile_pool(name="mlp", bufs=1)
            pmlp = mlp_pool_cm.__enter__()

            # =============================================================
            # Phase D: out-proj (token-split), residual, LN2 stats
            # =============================================================
            o_own = [pmlp.tile([128, 512], bf16, name=f"oo_{kk}", tag=f"oo_{kk}")
                     for kk in range(8)]
            for kk in range(8):
                nc.sync.dma_start(out=o_own[kk][0:64, :], in_=cc_out[0][kk])
                nc.sync.dma_start(out=o_own[kk][64:128, :], in_=cc_out[1][kk])

            xa = [pmlp.tile([128, 512], f32, name=f"xa_{m}", tag=f"xa_{m}") for m in range(8)]
            xab = [pmlp.tile([128, 512], bf16, name=f"xab_{m}", tag=f"xab_{m}") for m in range(8)]
            with tc.tile_pool(name="psOP", bufs=2, space="PSUM") as psOP, \
                 tc.tile_pool(name="psMU", bufs=1, space="PSUM") as psMU, \
                 tc.tile_pool(name="psSQ", bufs=1, space="PSUM") as psSQ, \
                 tc.tile_pool(name="psRB", bufs=1, space="PSUM") as psRB:
                ps_mu = psMU.tile([1, 512], f32)
                ps_sq = psSQ.tile([1, 512], f32)
                for m in range(8):
                    ps = psOP.tile([128, 512], f32, tag="op")
                    for kk in range(8):
                        nc.tensor.matmul(ps, wo_sb[m][:, kk, :], o_own[kk],
                                         start=(kk == 0), stop=(kk == 7))
                    nc.vector.tensor_add(out=xa[m], in0=ps, in1=xT_o[m])
                    nc.vector.tensor_copy(out=xab[m], in_=xa[m])
                    sq = pwork.tile([128, 512], bf16, tag="sq", bufs=2)
                    nc.vector.tensor_mul(out=sq, in0=xab[m], in1=xab[m])
                    nc.tensor.matmul(ps_mu, ones_col, xab[m],
                                     start=(m == 0), stop=(m == 7))
                    nc.tensor.matmul(ps_sq, ones_col, sq,
                                     start=(m == 0), stop=(m == 7))
                    # fold the mlp2 bias into the residual now (stats above
                    # already consumed the pre-bias value via xab/sq)
                    nc.scalar.activation(out=xa[m], in_=xa[m], func=AF.Identity,
                                         bias=b2_sb[m])

                # LN2 row stats: mu = sum/1024, var = sqsum/1024 - mu^2
                mu2 = pwork.tile([1, 512], f32, tag="mu2", bufs=1)
                nc.vector.tensor_scalar_mul(out=mu2, in0=ps_mu, scalar1=1.0 / D)
                mus_r = pwork.tile([1, 512], f32, tag="mus2_r", bufs=1)
                nc.vector.tensor_mul(out=mus_r, in0=mu2, in1=mu2)
                var_r = pwork.tile([1, 512], f32, tag="var2_r", bufs=1)
                nc.vector.scalar_tensor_tensor(
                    out=var_r, in0=ps_sq, scalar=1.0 / D, in1=mus_r,
                    op0=OP.mult, op1=OP.subtract)
                sd2 = pwork.tile([1, 512], f32, tag="sd2_r", bufs=1)
                nc.scalar.activation(out=sd2, in_=var_r, func=AF.Sqrt, bias=eps1)
                rstd2 = pwork.tile([1, 512], bf16, tag="rstd2_r", bufs=1)
                nc.vector.reciprocal(out=rstd2, in_=sd2)
                ps_rb = psRB.tile([128, 512], f32)
                nc.tensor.matmul(ps_rb, ones_row[0:1, 0:128], rstd2, start=True, stop=True)
                rstd2_sb = pmlp.tile([128, 512], bf16)
                nc.vector.tensor_copy(out=rstd2_sb, in_=ps_rb)

            # =============================================================
            # Phase E: MLP (token-split, full weights)
            # =============================================================
            u_g = [pmlp.tile([128, 512], bf16, name=f"ug_{m}", tag=f"ug_{m}") for m in range(32)]
            with tc.tile_pool(name="psU", bufs=3, space="PSUM") as psU, \
                 tc.tile_pool(name="psDn", bufs=2, space="PSUM") as psDn:
                for m in range(32):
                    ps = psU.tile([128, 512], f32, tag="u")
                    w = pstream.tile([128, 8, 128], bf16, tag="w1_st", bufs=4)
                    nc.sync.dma_start(out=w[:, 0:4, :], in_=w1_blk[m, :, 0:4, :])
                    nc.sync.dma_start(out=w[:, 4:8, :], in_=w1_blk[m, :, 4:8, :])
                    for kk in range(8):
                        nc.tensor.matmul(ps, w[:, kk, :], xab[kk],
                                         start=(kk == 0), stop=(kk == 7))
                    upre = pwork.tile([128, 512], bf16, tag="upre", bufs=2)
                    nc.vector.tensor_mul(out=upre, in0=ps, in1=rstd2_sb)
                    b1 = pwork.tile([128, 1], f32, tag="b1_st")
                    nc.sync.dma_start(out=b1, in_=b1_t[m])
                    nc.scalar.activation(out=u_g[m], in_=upre,
                                         func=AF.Gelu_apprx_tanh, bias=b1)
                for m in range(8):
                    ps = psDn.tile([128, 512], f32, tag="dn")
                    w = pstream.tile([128, 32, 128], bf16, tag="w2_st", bufs=2)
                    for q4 in range(4):
                        nc.sync.dma_start(out=w[:, q4 * 8:(q4 + 1) * 8, :],
                                          in_=w2_blk[m, :, q4 * 8:(q4 + 1) * 8, :])
                    for kk in range(32):
                        nc.tensor.matmul(ps, w[:, kk, :], u_g[kk],
                                         start=(kk == 0), stop=(kk == 31))
                    ot = pwork.tile([128, 512], f32, tag="ot", bufs=2)
                    nc.vector.tensor_add(out=ot, in0=ps, in1=xa[m])
                    nc.sync.dma_start(out=out_d[m * 128:(m + 1) * 128, :], in_=ot)
            mlp_pool_cm.__exit__(None, None, None)

    _legalize_sync(nc)
    return nc


# ---------------------------------------------------------------------------
# Host-side prep + execution
# ---------------------------------------------------------------------------
_NC_CACHE = {}


def _get_nc():
    if "nc" not in _NC_CACHE:
        _NC_CACHE["nc"] = _build()
    return _NC_CACHE["nc"]


def _bf(a):
    return np.ascontiguousarray(a).astype(ml_dtypes.bfloat16)


def _f32(a):
    return np.ascontiguousarray(a, dtype=np.float32)


def _mean_fold(w):
    # W' = W - rowsum(W)/D : folds LN mean-centering into the matmul
    return w - w.sum(1, keepdims=True) / w.shape[1]


def _prep_inputs(x, rot_cos, rot_sin, ln1_w, w_qkv, w_out, ln2_w, w_mlp1,
                 b_mlp1, w_mlp2, b_mlp2):
    x = np.asarray(x, np.float32)
    X = x.reshape(T, D)

    xT = X.T  # (D, T)
    # (ch, p, kk, t): partition row p holds all kk-blocks contiguously
    xT_blk = _bf(xT.reshape(8, 128, NCH, 512).transpose(2, 1, 0, 3))

    # rope tables: (128 rows = 2 heads x [first32|last32]) x T tokens
    cos = np.asarray(rot_cos, np.float32)[0, :, 0, 0, :HD // 2]  # (S, 32)
    sin = np.asarray(rot_sin, np.float32)[0, :, 0, 0, :HD // 2]
    cT = np.concatenate([cos, cos], 1).T          # (64, S)
    sT = np.concatenate([-sin, sin], 1).T         # (64, S) sign-folded
    cT = np.tile(cT, (2, B))                      # (128, T)
    sT = np.tile(sT, (2, B))
    tab = _bf(np.stack([cT, sT]))

    wqkv_eff = np.asarray(w_qkv, np.float32) * np.asarray(ln1_w, np.float32)[None, :]
    w1_eff = np.asarray(w_mlp1, np.float32) * np.asarray(ln2_w, np.float32)[None, :]
    w1_eff = _mean_fold(w1_eff)
    w_out_f = np.asarray(w_out, np.float32)
    w2_f = np.asarray(w_mlp2, np.float32)

    woutT = w_out_f.T  # (d_in=head dims, e)
    wout_blk = _bf(woutT.reshape(8, 128, 8, 128).transpose(2, 1, 0, 3))  # [m, p, kk, e]
    w1T = w1_eff.T     # (D, 4D)
    w1_blk = _bf(w1T.reshape(8, 128, 32, 128).transpose(2, 1, 0, 3))
    w2T = w2_f.T       # (4D, D)
    w2_blk = _bf(w2T.reshape(32, 128, 8, 128).transpose(2, 1, 0, 3))
    b1_arr = _f32(np.asarray(b_mlp1, np.float32).reshape(32, 128, 1))
    b2_arr = _f32(np.asarray(b_mlp2, np.float32).reshape(8, 128, 1))

    in_maps = []
    for c in range(N_CORES):
        w_sl = np.concatenate(
            [wqkv_eff[0 * D + 2 * c * HD: 0 * D + 2 * (c + 1) * HD] * 0.125,
             wqkv_eff[1 * D + 2 * c * HD: 1 * D + 2 * (c + 1) * HD],
             wqkv_eff[2 * D + 2 * c * HD: 2 * D + 2 * (c + 1) * HD]], 0)  # (384, D)
        w_sl = _mean_fold(w_sl)
        wT_sl = w_sl.T  # (D, 384) -> [m, p, kk, e]
        wqkv_b = _bf(wT_sl.reshape(8, 128, 3, 128).transpose(2, 1, 0, 3))
        in_maps.append({
            "xT_blk": xT_blk,
            "xT_own": _f32(xT[:, c * TOK:(c + 1) * TOK]),
            "wqkv_blk": wqkv_b,
            "tab": tab,
            "wout_blk": wout_blk,
            "w1_blk": w1_blk,
            "b1_t": b1_arr,
            "w2_blk": w2_blk,
            "b2_t": b2_arr,
        })
    return in_maps


def _assemble(results):
    outT = np.concatenate([results[c]["out"] for c in range(N_CORES)], axis=1)
    return np.ascontiguousarray(outT.T.astype(np.float32)).reshape(B, S, D)


def run_spmd(in_maps, **kwargs):
    nc = _get_nc()
    return run_bass_kernel_spmd(nc, in_maps, core_ids=list(range(N_CORES)), **kwargs)


def kernel(x, rot_cos, rot_sin, ln1_w, w_qkv, w_out, ln2_w, w_mlp1, b_mlp1,
           w_mlp2, b_mlp2):
    in_maps = _prep_inputs(x, rot_cos, rot_sin, ln1_w, w_qkv, w_out, ln2_w,
                           w_mlp1, b_mlp1, w_mlp2, b_mlp2)
    res = run_spmd(in_maps)
    return _assemble(res.results)


# revision 56
# speedup vs baseline: 1.1846x; 1.0032x over previous
"""Fused DDiT transformer block (causal) on 8 TRN2 NeuronCores.

Sharding: attention is head-parallel (2 heads/core, 16 total) with QKV
column-sliced per core; two AllToAlls (one per local head) re-shard from
head-split to token-split, and out-proj + MLP run token-parallel
(512 tokens/core). LayerNorm gains AND mean-centering are folded into the
following matmul weights on the host (W' = W - rowsum(W)/D), so only the
1/std factor is computed on device: each core computes LN1 sigma for its
own 512 tokens and an AllGather shares it; sigma folds into the RoPE
tables (q,k), a PSUM-eviction multiply (v), or a broadcast multiply (MLP).
Compute dtype bf16 (fp32 accumulation); the residual stream stays fp32.
"""
import sys

for _p in ("/opt/trn_rl_repo",):
    if _p not in sys.path:
        sys.path.append(_p)

import numpy as np
import ml_dtypes

import concourse.bass as bass
import concourse.tile as tile
import concourse.mybir as mybir
from concourse.bass_utils import run_bass_kernel_spmd
from concourse.masks import make_identity

bf16 = mybir.dt.bfloat16
f32 = mybir.dt.float32
AF = mybir.ActivationFunctionType
OP = mybir.AluOpType

N_CORES = 8
B, S, D = 2, 2048, 1024
T = B * S            # 4096 tokens total
NH, HD = 16, 64      # heads, head dim
HPC = NH // N_CORES  # 2 heads per core
TOK = T // N_CORES   # 512 tokens per core in the token-split phase
NT = T // 128        # 32 token tiles of 128
NCH = T // 512       # 8 chunks of 512 tokens
LN_EPS = 1e-5

# ---------------------------------------------------------------------------
# Sync legalizer: this walrus build accepts only ONE sync wait and ONE sync
# update per TPB instruction. Move extras onto same-engine NoOps (engines
# complete instructions in program order, so semantics are preserved).
# ---------------------------------------------------------------------------
_uid = [0]


def _legalize_sync(nc):
    for f in nc.m.functions:
        for bb in f.blocks:
            out = []
            changed = False
            for inst in bb.instructions:
                si = inst.sync_info
                if si is None:
                    out.append(inst)
                    continue
                waits = list(si.on_wait) if si.on_wait else []
                updates = list(si.on_update) if si.on_update else []
                if len(waits) <= 1 and len(updates) <= 1:
                    out.append(inst)
                    continue
                changed = True
                for w in waits[:-1]:
                    _uid[0] += 1
                    nop = mybir.InstNoOp(name=f"syncw-{_uid[0]}", ins=[], outs=[])
                    nop.engine = inst.engine
                    nop.sync_info = mybir.SyncInfo(on_wait=[w], on_update=[])
                    out.append(nop)
                inst.sync_info = mybir.SyncInfo(
                    on_wait=waits[-1:], on_update=updates[:1]
                )
                out.append(inst)
                for u in updates[1:]:
                    _uid[0] += 1
                    nop = mybir.InstNoOp(name=f"syncu-{_uid[0]}", ins=[], outs=[])
                    nop.engine = inst.engine
                    nop.sync_info = mybir.SyncInfo(on_wait=[], on_update=[u])
                    out.append(nop)
            if changed:
                bb.instructions = out
    return nc


# ---------------------------------------------------------------------------
# Kernel graph
# ---------------------------------------------------------------------------
def _build():
    nc = bass.Bass()

    # -- external inputs (per core)
    xT_blk = nc.dram_tensor("xT_blk", (NCH, 128, 8, 512), bf16, kind="ExternalInput")
    xT_own = nc.dram_tensor("xT_own", (D, TOK), f32, kind="ExternalInput")
    wqkv_blk = nc.dram_tensor("wqkv_blk", (3, 128, 8, 128), bf16, kind="ExternalInput")
    tab = nc.dram_tensor("tab", (2, 128, T), bf16, kind="ExternalInput")  # cos, sin(signed)
    wout_blk = nc.dram_tensor("wout_blk", (8, 128, 8, 128), bf16, kind="ExternalInput")
    w1_blk = nc.dram_tensor("w1_blk", (32, 128, 8, 128), bf16, kind="ExternalInput")
    b1_t = nc.dram_tensor("b1_t", (32, 128, 1), f32, kind="ExternalInput")
    w2_blk = nc.dram_tensor("w2_blk", (8, 128, 32, 128), bf16, kind="ExternalInput")
    b2_t = nc.dram_tensor("b2_t", (8, 128, 1), f32, kind="ExternalInput")
    sel2_d = nc.dram_tensor("sel2_d", (2, 128), bf16, kind="ExternalInput")
    out_d = nc.dram_tensor("out", (D, TOK), f32, kind="ExternalOutput")

    # -- internal DRAM
    ag_in = nc.dram_tensor("ag_in", (1, TOK), bf16, kind="Internal")
    ag_out = nc.dram_tensor("ag_out", (N_CORES, TOK), bf16, kind="Internal")
    # 130 rows per slice: [o_h0(64) | sums_h0 | o_h1(64) | sums_h1] —
    # normalization happens after the AllToAll, not in the attention loop
    cc_in = nc.dram_tensor("cc_in", (N_CORES, 130, TOK), bf16, kind="Internal")
    cc_out = nc.dram_tensor("cc_out", (N_CORES, 130, TOK), bf16, kind="Internal")

    with tile.TileContext(nc) as tc, \
         nc.allow_low_precision(reason="bf16 block compute"):
        with tc.tile_pool(name="const", bufs=1) as pconst, \
             tc.tile_pool(name="persist", bufs=1) as pper, \
             tc.tile_pool(name="stream", bufs=2) as pstream, \
             tc.tile_pool(name="big2", bufs=2) as pbig2, \
             tc.tile_pool(name="work", bufs=3) as pwork:
            # ---- residual stream + stats inputs first: tensor engine's
            # first work (LN1 stats) depends on these DMAs.
            xT_o = [pper.tile([128, 512], f32, name=f"xo_{kk}", tag=f"xo_{kk}")
                    for kk in range(8)]
            for kk in range(8):
                nc.sync.dma_start(out=xT_o[kk], in_=xT_own[kk * 128:(kk + 1) * 128, :])

            ident_bf = pconst.tile([128, 128], bf16)
            make_identity(nc, ident_bf)
            # head-pair selector for the post-AllToAll denominator broadcast:
            # out[m,:] = rec[0,:] for m<64 else rec[1,:]
            sel2 = pconst.tile([2, 128], bf16)
            nc.sync.dma_start(out=sel2, in_=sel2_d[:, :])
            ones_row = pconst.tile([1, 128], bf16)
            nc.vector.memset(ones_row, 1.0)
            ones_col = pconst.tile([128, 1], bf16)
            nc.vector.memset(ones_col, 1.0)
            eps1 = pconst.tile([1, 1], f32)
            nc.vector.memset(eps1, LN_EPS)

            # =============================================================
            # Phase A: LN1 sigma for OWN 512 tokens; AllGather across cores
            # =============================================================
            rstd_row = pper.tile([1, T], bf16)
            with tc.tile_pool(name="psST", bufs=1, space="PSUM") as psST:
                ps_mu = psST.tile([1, 512], f32, tag="mu")
                ps_sq = psST.tile([1, 512], f32, tag="sq")
                for kk in range(8):
                    xb = pwork.tile([128, 512], bf16, tag="xb", bufs=2)
                    nc.scalar.activation(out=xb, in_=xT_o[kk], func=AF.Copy)
                    sqb = pwork.tile([128, 512], bf16, tag="sqb", bufs=2)
                    nc.vector.tensor_mul(out=sqb, in0=xb, in1=xb)
                    nc.tensor.matmul(ps_mu, ones_col, xb,
                                     start=(kk == 0), stop=(kk == 7))
                    nc.tensor.matmul(ps_sq, ones_col, sqb,
                                     start=(kk == 0), stop=(kk == 7))
                mu_r = pwork.tile([1, 512], f32, tag="mu_r", bufs=1)
                nc.vector.tensor_scalar_mul(out=mu_r, in0=ps_mu, scalar1=1.0 / D)
                mus_r = pwork.tile([1, 512], f32, tag="mus_r", bufs=1)
                nc.vector.tensor_mul(out=mus_r, in0=mu_r, in1=mu_r)
                var_r = pwork.tile([1, 512], f32, tag="var_r", bufs=1)
                nc.vector.scalar_tensor_tensor(
                    out=var_r, in0=ps_sq, scalar=1.0 / D, in1=mus_r,
                    op0=OP.mult, op1=OP.subtract)
                sd_own = pwork.tile([1, 512], f32, tag="sd_own", bufs=1)
                nc.scalar.activation(out=sd_own, in_=var_r, func=AF.Sqrt,
                                     bias=eps1)
                rstd_own = pwork.tile([1, 512], bf16, tag="rstd_own", bufs=1)
                nc.vector.reciprocal(out=rstd_own, in_=sd_own)
                nc.sync.dma_start(out=ag_in[0:1, :], in_=rstd_own)
                nc.gpsimd.collective_compute(
                    "AllGather", OP.bypass, ins=[ag_in[:, :]], outs=[ag_out[:, :]],
                    replica_groups=[list(range(N_CORES))])
                for c in range(N_CORES):
                    nc.sync.dma_start(out=rstd_row[0:1, c * 512:(c + 1) * 512],
                                      in_=ag_out[c:c + 1, :])

            # ---- weights / tables (prefetch while stats run); phase-B-only
            # tensors live in the attention-scoped pool so MLP gets the space
            attn_pool_cm = tc.tile_pool(name="attn", bufs=1)
            pattn = attn_pool_cm.__enter__()
            # full-width causal masks for the 4 diagonal offsets: for a k-tile
            # at columns [off, off+128) keep where q - k = f - p - off >= 0
            maskfull = []
            for oi in range(4):
                mf = pattn.tile([128, 512], bf16, name=f"mask{oi}", tag=f"mask{oi}")
                nc.gpsimd.memset(mf, 1.0)
                nc.gpsimd.affine_select(
                    out=mf, in_=mf, pattern=[[1, 512]],
                    compare_op=OP.is_ge, fill=0.0, base=-oi * 128,
                    channel_multiplier=-1)
                maskfull.append(mf)
            wq_sb = {}
            for m in range(3):
                w = pattn.tile([128, 8, 128], bf16, name=f"wqkv_{m}", tag=f"wqkv_{m}")
                nc.sync.dma_start(out=w, in_=wqkv_blk[m])
                wq_sb[m] = w
            # wout/b2 tiles allocated now, DMA'd during attention (phase C)
            wo_sb = {m: pconst.tile([128, 8, 128], bf16, name=f"wout_{m}",
                                    tag=f"wout_{m}") for m in range(8)}
            b2_sb = {m: pconst.tile([128, 1], f32, name=f"b2_{m}", tag=f"b2_{m}")
                     for m in range(8)}

            # =============================================================
            # Phase B: QKV projection + RoPE + V transpose (pipelined)
            # =============================================================
            qT_sb = pattn.tile([128, T], bf16)
            kT_sb = pattn.tile([128, T], bf16)
            v_all = pattn.tile([128, NT, 130], bf16)

            # ---- QKV mains for ALL chunks first: none of this needs the
            # AllGather'd rstd, so the PE stays busy through the collective.
            # Raw QKV is evicted to SBUF via the (otherwise idle) scalar engine.
            raws = {}
            with tc.tile_pool(name="psQKV", bufs=6, space="PSUM") as psQ:
                for ch in range(NCH):
                    xrt = pbig2.tile([128, 8, 512], bf16, tag="xTr")
                    nc.sync.dma_start(out=xrt, in_=xT_blk[ch])
                    rs = []
                    for m in range(3):
                        ps = psQ.tile([128, 512], f32, tag="qkv")
                        for kk in range(8):
                            nc.tensor.matmul(ps, wq_sb[m][:, kk, :], xrt[:, kk, :],
                                             start=(kk == 0), stop=(kk == 7))
                        r = pattn.tile([128, 512], bf16, name=f"raw_{ch}_{m}",
                                       tag=f"raw_{ch}_{m}")
                        nc.scalar.activation(out=r, in_=ps, func=AF.Copy)
                        rs.append(r)
                    raws[ch] = rs

            # out-proj weights/bias stream in while attention computes
            for m in range(8):
                nc.sync.dma_start(out=wo_sb[m][:, 0:4, :], in_=wout_blk[m, :, 0:4, :])
                nc.sync.dma_start(out=wo_sb[m][:, 4:8, :], in_=wout_blk[m, :, 4:8, :])
                nc.sync.dma_start(out=b2_sb[m], in_=b2_t[m])

            nc.vector.memset(v_all[:, :, 64:65], 1.0)
            nc.vector.memset(v_all[:, :, 129:130], 1.0)

            with tc.tile_pool(name="psVT", bufs=1, space="PSUM") as psVT, \
                 tc.tile_pool(name="psB", bufs=1, space="PSUM") as psB, \
                 tc.tile_pool(name="psSC", bufs=2, space="PSUM") as psSC, \
                 tc.tile_pool(name="psO", bufs=1, space="PSUM") as psO:

                def bcastfold(ch):
                    sl = slice(ch * 512, (ch + 1) * 512)
                    ps_b = psB.tile([128, 512], f32, tag="bc")
                    nc.tensor.matmul(ps_b, ones_row[0:1, 0:128], rstd_row[0:1, sl],
                                     start=True, stop=True)
                    rstd_sb = pwork.tile([128, 512], bf16, tag="rstd", bufs=2)
                    nc.scalar.activation(out=rstd_sb, in_=ps_b, func=AF.Copy)
                    # rope tables stream per chunk; rstd folds in place
                    tab_c = pwork.tile([128, 512], bf16, tag="tabc", bufs=2)
                    nc.sync.dma_start(out=tab_c, in_=tab[0, :, sl])
                    tab_s = pwork.tile([128, 512], bf16, tag="tabs", bufs=2)
                    nc.sync.dma_start(out=tab_s, in_=tab[1, :, sl])
                    nc.vector.tensor_mul(out=tab_c, in0=tab_c, in1=rstd_sb)
                    nc.vector.tensor_mul(out=tab_s, in0=tab_s, in1=rstd_sb)
                    return rstd_sb, tab_c, tab_s

                def evict(ch, fold):
                    rstd_sb, tab_c, tab_s = fold
                    sl = slice(ch * 512, (ch + 1) * 512)
                    for m in range(2):  # q, k: rope from the raw SBUF copy
                        raw = raws[ch][m]
                        dst = qT_sb if m == 0 else kT_sb
                        tsw = pwork.tile([128, 512], bf16, tag="ropesw", bufs=2)
                        for h in range(2):
                            for a2 in range(2):
                                nc.vector.tensor_copy(
                                    out=tsw[h * 64 + a2 * 32:h * 64 + a2 * 32 + 32, :],
                                    in_=raw[h * 64 + (1 - a2) * 32:h * 64 + (1 - a2) * 32 + 32, :])
                        t1 = pwork.tile([128, 512], bf16, tag="ropet1", bufs=2)
                        nc.vector.tensor_mul(out=t1, in0=raw, in1=tab_c)
                        t2 = pwork.tile([128, 512], bf16, tag="ropet2", bufs=2)
                        nc.vector.tensor_mul(out=t2, in0=tsw, in1=tab_s)
                        nc.vector.tensor_add(out=dst[:, sl], in0=t1, in1=t2)
                    # v: scale by rstd, transpose to [t, e] tiles
                    vt = pwork.tile([128, 512], bf16, tag="vtmp")
                    nc.vector.tensor_mul(out=vt, in0=raws[ch][2], in1=rstd_sb)
                    for j in range(4):
                        g = ch * 4 + j
                        pst = psVT.tile([128, 128], bf16, tag="vtr")
                        nc.tensor.transpose(out=pst, in_=vt[:, j * 128:(j + 1) * 128],
                                            identity=ident_bf)
                        nc.vector.tensor_copy(out=v_all[:, g, 0:64], in_=pst[:, 0:64])
                        nc.vector.tensor_copy(out=v_all[:, g, 65:129], in_=pst[:, 64:128])

                def attn_batch(b):
                    # both heads per k-tile: the score matmuls run CONCURRENTLY
                    # in disjoint PE row-halves via tile_position, so the array
                    # sees full activity (keeps the HAM clock-gate warm)
                    for qc in range(4):
                        qsl = slice(b * 2048 + qc * 512, b * 2048 + (qc + 1) * 512)
                        nkt = 4 * (qc + 1)
                        ps_o0 = psO.tile([65, 512], f32, tag="o0")
                        ps_o1 = psO.tile([65, 512], f32, tag="o1")
                        p_prev = None
                        for kt in range(nkt):
                            ksl = slice(b * 2048 + kt * 128, b * 2048 + (kt + 1) * 128)
                            ps_s0 = psSC.tile([128, 512], f32, tag="sc0")
                            ps_s1 = psSC.tile([128, 512], f32, tag="sc1")
                            nc.tensor.matmul(ps_s0, kT_sb[0:64, ksl], qT_sb[0:64, qsl],
                                             start=True, stop=True,
                                             tile_position=(0, 0))
                            nc.tensor.matmul(ps_s1, kT_sb[64:128, ksl],
                                             qT_sb[64:128, qsl],
                                             start=True, stop=True,
                                             tile_position=(64, 0))
                            p0 = pwork.tile([128, 512], bf16, tag="p0", bufs=4)
                            nc.scalar.activation(out=p0, in_=ps_s0, func=AF.Exp)
                            p1 = pwork.tile([128, 512], bf16, tag="p1", bufs=4)
                            nc.scalar.activation(out=p1, in_=ps_s1, func=AF.Exp)
                            if kt >= 4 * qc:  # diagonal block: causal mask
                                mf = maskfull[kt - 4 * qc]
                                nc.gpsimd.tensor_mul(out=p0, in0=p0, in1=mf)
                                nc.gpsimd.tensor_mul(out=p1, in0=p1, in1=mf)
                            if p_prev is not None:
                                g, q0, q1, first = p_prev
                                nc.tensor.matmul(ps_o0, v_all[:, g, 0:65], q0,
                                                 start=first, stop=False)
                                nc.tensor.matmul(ps_o1, v_all[:, g, 65:130], q1,
                                                 start=first, stop=False)
                            p_prev = (b * 16 + kt, p0, p1, kt == 0)
                        g, q0, q1, first = p_prev
                        nc.tensor.matmul(ps_o0, v_all[:, g, 0:65], q0,
                                         start=first, stop=True)
                        nc.tensor.matmul(ps_o1, v_all[:, g, 65:130], q1,
                                         start=first, stop=True)
                        # ship UNNORMALIZED o + sums; divide after the AllToAll
                        j = b * 4 + qc
                        o_u0 = pwork.tile([65, 512], bf16, tag="o_u0", bufs=2)
                        nc.vector.tensor_copy(out=o_u0, in_=ps_o0)
                        nc.sync.dma_start(out=cc_in[j, 0:65, :], in_=o_u0)
                        o_u1 = pwork.tile([65, 512], bf16, tag="o_u1", bufs=2)
                        nc.vector.tensor_copy(out=o_u1, in_=ps_o1)
                        nc.sync.dma_start(out=cc_in[j, 65:130, :], in_=o_u1)

                for ch in range(4):
                    evict(ch, bcastfold(ch))
                attn_batch(0)
                for ch in range(4, NCH):
                    evict(ch, bcastfold(ch))
                attn_batch(1)

            nc.gpsimd.collective_compute(
                "AllToAll", OP.bypass, ins=[cc_in[:, :, :]], outs=[cc_out[:, :, :]],
                replica_groups=[list(range(N_CORES))])

            attn_pool_cm.__exit__(None, None, None)
            mlp_pool_cm = tc.tile_pool(name="mlp", bufs=1)
            pmlp = mlp_pool_cm.__enter__()

            # =============================================================
            # Phase D: out-proj (token-split), residual, LN2 stats
            # =============================================================
            pD_cm = tc.tile_pool(name="pD", bufs=1)
            pD = pD_cm.__enter__()
            o_own = [pD.tile([128, 512], bf16, name=f"oo_{kk}", tag=f"oo_{kk}")
                     for kk in range(8)]
            den_all = pD.tile([16, 512], bf16)
            for kk in range(8):
                nc.sync.dma_start(out=o_own[kk][0:64, :], in_=cc_out[kk, 0:64, :])
                nc.sync.dma_start(out=o_own[kk][64:128, :], in_=cc_out[kk, 65:129, :])
                nc.sync.dma_start(out=den_all[2 * kk:2 * kk + 1, :],
                                  in_=cc_out[kk, 64:65, :])
                nc.sync.dma_start(out=den_all[2 * kk + 1:2 * kk + 2, :],
                                  in_=cc_out[kk, 129:130, :])
            # ONE batched in-place reciprocal for all 16 denominator rows;
            # pairs are pulled to base partition 0 by tiny SBUF DMAs, then
            # broadcast (sel2) + in-place multiply normalizes o_own
            nc.vector.reciprocal(out=den_all, in_=den_all)
            with tc.tile_pool(name="psNB", bufs=2, space="PSUM") as psNB:
                for kk in range(8):
                    pair = pwork.tile([2, 512], bf16, tag="rpair", bufs=2)
                    nc.sync.dma_start(out=pair, in_=den_all[2 * kk:2 * kk + 2, :])
                    ps_nb = psNB.tile([128, 512], f32, tag="nb")
                    nc.tensor.matmul(ps_nb, sel2, pair, start=True, stop=True)
                    nc.vector.tensor_mul(out=o_own[kk], in0=o_own[kk], in1=ps_nb)

            xa = [pmlp.tile([128, 512], f32, name=f"xa_{m}", tag=f"xa_{m}") for m in range(8)]
            xab = [pmlp.tile([128, 512], bf16, name=f"xab_{m}", tag=f"xab_{m}") for m in range(8)]
            with tc.tile_pool(name="psOP", bufs=2, space="PSUM") as psOP, \
                 tc.tile_pool(name="psMU", bufs=1, space="PSUM") as psMU, \
                 tc.tile_pool(name="psSQ", bufs=1, space="PSUM") as psSQ, \
                 tc.tile_pool(name="psRB", bufs=1, space="PSUM") as psRB:
                ps_mu = psMU.tile([1, 512], f32)
                ps_sq = psSQ.tile([1, 512], f32)
                for m in range(8):
                    ps = psOP.tile([128, 512], f32, tag="op")
                    for kk in range(8):
                        nc.tensor.matmul(ps, wo_sb[m][:, kk, :], o_own[kk],
                                         start=(kk == 0), stop=(kk == 7))
                    nc.vector.tensor_add(out=xa[m], in0=ps, in1=xT_o[m])
                    nc.vector.tensor_copy(out=xab[m], in_=xa[m])
                    sq = pwork.tile([128, 512], bf16, tag="sq", bufs=2)
                    nc.vector.tensor_mul(out=sq, in0=xab[m], in1=xab[m])
                    nc.tensor.matmul(ps_mu, ones_col, xab[m],
                                     start=(m == 0), stop=(m == 7))
                    nc.tensor.matmul(ps_sq, ones_col, sq,
                                     start=(m == 0), stop=(m == 7))
                    # fold the mlp2 bias into the residual now (stats above
                    # already consumed the pre-bias value via xab/sq)
                    nc.scalar.activation(out=xa[m], in_=xa[m], func=AF.Identity,
                                         bias=b2_sb[m])

                # LN2 row stats: mu = sum/1024, var = sqsum/1024 - mu^2
                mu2 = pwork.tile([1, 512], f32, tag="mu2", bufs=1)
                nc.vector.tensor_scalar_mul(out=mu2, in0=ps_mu, scalar1=1.0 / D)
                mus_r = pwork.tile([1, 512], f32, tag="mus2_r", bufs=1)
                nc.vector.tensor_mul(out=mus_r, in0=mu2, in1=mu2)
                var_r = pwork.tile([1, 512], f32, tag="var2_r", bufs=1)
                nc.vector.scalar_tensor_tensor(
                    out=var_r, in0=ps_sq, scalar=1.0 / D, in1=mus_r,
                    op0=OP.mult, op1=OP.subtract)
                sd2 = pwork.tile([1, 512], f32, tag="sd2_r", bufs=1)
                nc.scalar.activation(out=sd2, in_=var_r, func=AF.Sqrt, bias=eps1)
                rstd2 = pwork.tile([1, 512], bf16, tag="rstd2_r", bufs=1)
                nc.vector.reciprocal(out=rstd2, in_=sd2)
                ps_rb = psRB.tile([128, 512], f32)
                nc.tensor.matmul(ps_rb, ones_row[0:1, 0:128], rstd2, start=True, stop=True)
                rstd2_sb = pmlp.tile([128, 512], bf16)
                nc.vector.tensor_copy(out=rstd2_sb, in_=ps_rb)
            pD_cm.__exit__(None, None, None)

            # =============================================================
            # Phase E: MLP (token-split, full weights)
            # =============================================================
            u_g = [pmlp.tile([128, 512], bf16, name=f"ug_{m}", tag=f"ug_{m}") for m in range(32)]
            with tc.tile_pool(name="psU", bufs=3, space="PSUM") as psU, \
                 tc.tile_pool(name="psDn", bufs=2, space="PSUM") as psDn:
                for m in range(32):
                    ps = psU.tile([128, 512], f32, tag="u")
                    w = pstream.tile([128, 8, 128], bf16, tag="w1_st", bufs=4)
                    nc.sync.dma_start(out=w[:, 0:4, :], in_=w1_blk[m, :, 0:4, :])
                    nc.sync.dma_start(out=w[:, 4:8, :], in_=w1_blk[m, :, 4:8, :])
                    for kk in range(8):
                        nc.tensor.matmul(ps, w[:, kk, :], xab[kk],
                                         start=(kk == 0), stop=(kk == 7))
                    upre = pwork.tile([128, 512], bf16, tag="upre", bufs=2)
                    nc.vector.tensor_mul(out=upre, in0=ps, in1=rstd2_sb)
                    b1 = pwork.tile([128, 1], f32, tag="b1_st")
                    nc.sync.dma_start(out=b1, in_=b1_t[m])
                    nc.scalar.activation(out=u_g[m], in_=upre,
                                         func=AF.Gelu_apprx_tanh, bias=b1)
                for m in range(8):
                    ps = psDn.tile([128, 512], f32, tag="dn")
                    w = pstream.tile([128, 32, 128], bf16, tag="w2_st", bufs=2)
                    for q4 in range(4):
                        nc.sync.dma_start(out=w[:, q4 * 8:(q4 + 1) * 8, :],
                                          in_=w2_blk[m, :, q4 * 8:(q4 + 1) * 8, :])
                    for kk in range(32):
                        nc.tensor.matmul(ps, w[:, kk, :], u_g[kk],
                                         start=(kk == 0), stop=(kk == 31))
                    ot = pwork.tile([128, 512], f32, tag="ot", bufs=2)
                    nc.vector.tensor_add(out=ot, in0=ps, in1=xa[m])
                    nc.sync.dma_start(out=out_d[m * 128:(m + 1) * 128, :], in_=ot)
            mlp_pool_cm.__exit__(None, None, None)

    _legalize_sync(nc)
    return nc


# ---------------------------------------------------------------------------
# Host-side prep + execution
# ---------------------------------------------------------------------------
_NC_CACHE = {}


def _get_nc():
    if "nc" not in _NC_CACHE:
        _NC_CACHE["nc"] = _build()
    return _NC_CACHE["nc"]


def _bf(a):
    return np.ascontiguousarray(a).astype(ml_dtypes.bfloat16)


def _f32(a):
    return np.ascontiguousarray(a, dtype=np.float32)


def _mean_fold(w):
    # W' = W - rowsum(W)/D : folds LN mean-centering into the matmul
    return w - w.sum(1, keepdims=True) / w.shape[1]


def _prep_inputs(x, rot_cos, rot_sin, ln1_w, w_qkv, w_out, ln2_w, w_mlp1,
                 b_mlp1, w_mlp2, b_mlp2):
    x = np.asarray(x, np.float32)
    X = x.reshape(T, D)

    xT = X.T  # (D, T)
    # (ch, p, kk, t): partition row p holds all kk-blocks contiguously
    xT_blk = _bf(xT.reshape(8, 128, NCH, 512).transpose(2, 1, 0, 3))

    # rope tables: (128 rows = 2 heads x [first32|last32]) x T tokens
    cos = np.asarray(rot_cos, np.float32)[0, :, 0, 0, :HD // 2]  # (S, 32)
    sin = np.asarray(rot_sin, np.float32)[0, :, 0, 0, :HD // 2]
    cT = np.concatenate([cos, cos], 1).T          # (64, S)
    sT = np.concatenate([-sin, sin], 1).T         # (64, S) sign-folded
    cT = np.tile(cT, (2, B))                      # (128, T)
    sT = np.tile(sT, (2, B))
    tab = _bf(np.stack([cT, sT]))

    wqkv_eff = np.asarray(w_qkv, np.float32) * np.asarray(ln1_w, np.float32)[None, :]
    w1_eff = np.asarray(w_mlp1, np.float32) * np.asarray(ln2_w, np.float32)[None, :]
    w1_eff = _mean_fold(w1_eff)
    w_out_f = np.asarray(w_out, np.float32)
    w2_f = np.asarray(w_mlp2, np.float32)

    woutT = w_out_f.T  # (d_in=head dims, e)
    wout_blk = _bf(woutT.reshape(8, 128, 8, 128).transpose(2, 1, 0, 3))  # [m, p, kk, e]
    w1T = w1_eff.T     # (D, 4D)
    w1_blk = _bf(w1T.reshape(8, 128, 32, 128).transpose(2, 1, 0, 3))
    w2T = w2_f.T       # (4D, D)
    w2_blk = _bf(w2T.reshape(32, 128, 8, 128).transpose(2, 1, 0, 3))
    b1_arr = _f32(np.asarray(b_mlp1, np.float32).reshape(32, 128, 1))
    b2_arr = _f32(np.asarray(b_mlp2, np.float32).reshape(8, 128, 1))
    sel2 = np.zeros((2, 128), np.float32)
    sel2[0, 0:64] = 1.0
    sel2[1, 64:128] = 1.0
    sel2 = _bf(sel2)

    in_maps = []
    for c in range(N_CORES):
        w_sl = np.concatenate(
            [wqkv_eff[0 * D + 2 * c * HD: 0 * D + 2 * (c + 1) * HD] * 0.125,
             wqkv_eff[1 * D + 2 * c * HD: 1 * D + 2 * (c + 1) * HD],
             wqkv_eff[2 * D + 2 * c * HD: 2 * D + 2 * (c + 1) * HD]], 0)  # (384, D)
        w_sl = _mean_fold(w_sl)
        wT_sl = w_sl.T  # (D, 384) -> [m, p, kk, e]
        wqkv_b = _bf(wT_sl.reshape(8, 128, 3, 128).transpose(2, 1, 0, 3))
        in_maps.append({
            "xT_blk": xT_blk,
            "xT_own": _f32(xT[:, c * TOK:(c + 1) * TOK]),
            "wqkv_blk": wqkv_b,
            "tab": tab,
            "wout_blk": wout_blk,
            "w1_blk": w1_blk,
            "b1_t": b1_arr,
            "w2_blk": w2_blk,
            "b2_t": b2_arr,
            "sel2_d": sel2,
        })
    return in_maps


def _assemble(results):
    outT = np.concatenate([results[c]["out"] for c in range(N_CORES)], axis=1)
    return np.ascontiguousarray(outT.T.astype(np.float32)).reshape(B, S, D)


def run_spmd(in_maps, **kwargs):
    nc = _get_nc()
    return run_bass_kernel_spmd(nc, in_maps, core_ids=list(range(N_CORES)), **kwargs)


def kernel(x, rot_cos, rot_sin, ln1_w, w_qkv, w_out, ln2_w, w_mlp1, b_mlp1,
           w_mlp2, b_mlp2):
    in_maps = _prep_inputs(x, rot_cos, rot_sin, ln1_w, w_qkv, w_out, ln2_w,
                           w_mlp1, b_mlp1, w_mlp2, b_mlp2)
    res = run_spmd(in_maps)
    return _assemble(res.results)


# revision 64
# speedup vs baseline: 1.2340x; 1.0417x over previous
"""Fused DDiT transformer block (causal) on 8 TRN2 NeuronCores.

Sharding: attention is head-parallel (2 heads/core, 16 total) with QKV
column-sliced per core; two AllToAlls (one per local head) re-shard from
head-split to token-split, and out-proj + MLP run token-parallel
(512 tokens/core). LayerNorm gains AND mean-centering are folded into the
following matmul weights on the host (W' = W - rowsum(W)/D), so only the
1/std factor is computed on device: each core computes LN1 sigma for its
own 512 tokens and an AllGather shares it; sigma folds into the RoPE
tables (q,k), a PSUM-eviction multiply (v), or a broadcast multiply (MLP).
Compute dtype bf16 (fp32 accumulation); the residual stream stays fp32.
"""
import sys

for _p in ("/opt/trn_rl_repo",):
    if _p not in sys.path:
        sys.path.append(_p)

import numpy as np
import ml_dtypes

import concourse.bass as bass
import concourse.tile as tile
import concourse.mybir as mybir
from concourse.bass_utils import run_bass_kernel_spmd
from concourse.masks import make_identity

bf16 = mybir.dt.bfloat16
f32 = mybir.dt.float32
AF = mybir.ActivationFunctionType
OP = mybir.AluOpType

N_CORES = 8
B, S, D = 2, 2048, 1024
T = B * S            # 4096 tokens total
NH, HD = 16, 64      # heads, head dim
HPC = NH // N_CORES  # 2 heads per core
TOK = T // N_CORES   # 512 tokens per core in the token-split phase
NT = T // 128        # 32 token tiles of 128
NCH = T // 512       # 8 chunks of 512 tokens
LN_EPS = 1e-5

# ---------------------------------------------------------------------------
# Sync legalizer: this walrus build accepts only ONE sync wait and ONE sync
# update per TPB instruction. Move extras onto same-engine NoOps (engines
# complete instructions in program order, so semantics are preserved).
# ---------------------------------------------------------------------------
_uid = [0]


def _legalize_sync(nc):
    for f in nc.m.functions:
        for bb in f.blocks:
            out = []
            changed = False
            for inst in bb.instructions:
                si = inst.sync_info
                if si is None:
                    out.append(inst)
                    continue
                waits = list(si.on_wait) if si.on_wait else []
                updates = list(si.on_update) if si.on_update else []
                if len(waits) <= 1 and len(updates) <= 1:
                    out.append(inst)
                    continue
                changed = True
                for w in waits[:-1]:
                    _uid[0] += 1
                    nop = mybir.InstNoOp(name=f"syncw-{_uid[0]}", ins=[], outs=[])
                    nop.engine = inst.engine
                    nop.sync_info = mybir.SyncInfo(on_wait=[w], on_update=[])
                    out.append(nop)
                inst.sync_info = mybir.SyncInfo(
                    on_wait=waits[-1:], on_update=updates[:1]
                )
                out.append(inst)
                for u in updates[1:]:
                    _uid[0] += 1
                    nop = mybir.InstNoOp(name=f"syncu-{_uid[0]}", ins=[], outs=[])
                    nop.engine = inst.engine
                    nop.sync_info = mybir.SyncInfo(on_wait=[], on_update=[u])
                    out.append(nop)
            if changed:
                bb.instructions = out
    return nc


# ---------------------------------------------------------------------------
# Kernel graph
# ---------------------------------------------------------------------------
def _build():
    nc = bass.Bass()

    # -- external inputs (per core)
    xT_blk = nc.dram_tensor("xT_blk", (NCH, 128, 8, 512), bf16, kind="ExternalInput")
    xT_own = nc.dram_tensor("xT_own", (D, TOK), f32, kind="ExternalInput")
    wqkv_blk = nc.dram_tensor("wqkv_blk", (3, 128, 8, 128), bf16, kind="ExternalInput")
    tab = nc.dram_tensor("tab", (2, 128, T), bf16, kind="ExternalInput")  # cos, sin(signed)
    wout_blk = nc.dram_tensor("wout_blk", (8, 128, 8, 128), bf16, kind="ExternalInput")
    w1_blk = nc.dram_tensor("w1_blk", (32, 128, 8, 128), bf16, kind="ExternalInput")
    b1_t = nc.dram_tensor("b1_t", (32, 128, 1), f32, kind="ExternalInput")
    w2_blk = nc.dram_tensor("w2_blk", (8, 128, 32, 128), bf16, kind="ExternalInput")
    b2_t = nc.dram_tensor("b2_t", (8, 128, 1), f32, kind="ExternalInput")
    sel2_d = nc.dram_tensor("sel2_d", (16, 8 * 128), bf16, kind="ExternalInput")
    out_d = nc.dram_tensor("out", (D, TOK), f32, kind="ExternalOutput")

    # -- internal DRAM
    ag_in = nc.dram_tensor("ag_in", (1, TOK), bf16, kind="Internal")
    ag_out = nc.dram_tensor("ag_out", (N_CORES, TOK), bf16, kind="Internal")
    # 65 rows per slice: [o_h(64) | sums_h] — normalization happens after
    # the AllToAll, not in the attention loop
    cc_in = [nc.dram_tensor(f"cc_in{h}", (N_CORES, 65, TOK), bf16, kind="Internal")
             for h in range(2)]
    cc_out = [nc.dram_tensor(f"cc_out{h}", (N_CORES, 65, TOK), bf16, kind="Internal")
              for h in range(2)]
    warm_in = nc.dram_tensor("warm_in", (1, 16), bf16, kind="Internal")
    warm_out = nc.dram_tensor("warm_out", (N_CORES, 16), bf16, kind="Internal")

    with tile.TileContext(nc) as tc, \
         nc.allow_low_precision(reason="bf16 block compute"):
        with tc.tile_pool(name="const", bufs=1) as pconst, \
             tc.tile_pool(name="persist", bufs=1) as pper, \
             tc.tile_pool(name="stream", bufs=2) as pstream, \
             tc.tile_pool(name="big2", bufs=2) as pbig2, \
             tc.tile_pool(name="work", bufs=3) as pwork:
            # ---- residual stream + stats inputs first: tensor engine's
            # first work (LN1 stats) depends on these DMAs.
            xT_o = [pper.tile([128, 512], f32, name=f"xo_{kk}", tag=f"xo_{kk}")
                    for kk in range(8)]
            for kk in range(8):
                nc.sync.dma_start(out=xT_o[kk], in_=xT_own[kk * 128:(kk + 1) * 128, :])

            ident_bf = pconst.tile([128, 128], bf16)
            make_identity(nc, ident_bf)
            # absorb the one-time collective-infrastructure warmup (~25us)
            # behind the input DMAs: a dependency-free dummy AllGather
            nc.gpsimd.collective_compute(
                "AllGather", OP.bypass, ins=[warm_in[:, :]], outs=[warm_out[:, :]],
                replica_groups=[list(range(N_CORES))])
            # per-kk head selector for the post-AllToAll denominator broadcast:
            # col block kk: out[m,:] = rec[kk,:] for m<64 else rec[kk+8,:]
            sel16 = pconst.tile([16, 8 * 128], bf16)
            nc.sync.dma_start(out=sel16, in_=sel2_d[:, :])
            ones_row = pconst.tile([1, 128], bf16)
            nc.vector.memset(ones_row, 1.0)
            ones_col = pconst.tile([128, 1], bf16)
            nc.vector.memset(ones_col, 1.0)
            eps1 = pconst.tile([1, 1], f32)
            nc.vector.memset(eps1, LN_EPS)

            # =============================================================
            # Phase A: LN1 sigma for OWN 512 tokens; AllGather across cores
            # =============================================================
            rstd_row = pper.tile([1, T], bf16)
            with tc.tile_pool(name="psST", bufs=1, space="PSUM") as psST:
                ps_mu = psST.tile([1, 512], f32, tag="mu")
                ps_sq = psST.tile([1, 512], f32, tag="sq")
                for kk in range(8):
                    xb = pwork.tile([128, 512], bf16, tag="xb", bufs=2)
                    nc.scalar.activation(out=xb, in_=xT_o[kk], func=AF.Copy)
                    sqb = pwork.tile([128, 512], bf16, tag="sqb", bufs=2)
                    nc.vector.tensor_mul(out=sqb, in0=xb, in1=xb)
                    nc.tensor.matmul(ps_mu, ones_col, xb,
                                     start=(kk == 0), stop=(kk == 7))
                    nc.tensor.matmul(ps_sq, ones_col, sqb,
                                     start=(kk == 0), stop=(kk == 7))
                mu_r = pwork.tile([1, 512], f32, tag="mu_r", bufs=1)
                nc.vector.tensor_scalar_mul(out=mu_r, in0=ps_mu, scalar1=1.0 / D)
                mus_r = pwork.tile([1, 512], f32, tag="mus_r", bufs=1)
                nc.vector.tensor_mul(out=mus_r, in0=mu_r, in1=mu_r)
                var_r = pwork.tile([1, 512], f32, tag="var_r", bufs=1)
                nc.vector.scalar_tensor_tensor(
                    out=var_r, in0=ps_sq, scalar=1.0 / D, in1=mus_r,
                    op0=OP.mult, op1=OP.subtract)
                sd_own = pwork.tile([1, 512], f32, tag="sd_own", bufs=1)
                nc.scalar.activation(out=sd_own, in_=var_r, func=AF.Sqrt,
                                     bias=eps1)
                rstd_own = pwork.tile([1, 512], bf16, tag="rstd_own", bufs=1)
                nc.vector.reciprocal(out=rstd_own, in_=sd_own)
                nc.sync.dma_start(out=ag_in[0:1, :], in_=rstd_own)
                nc.gpsimd.collective_compute(
                    "AllGather", OP.bypass, ins=[ag_in[:, :]], outs=[ag_out[:, :]],
                    replica_groups=[list(range(N_CORES))])
                for c in range(N_CORES):
                    nc.sync.dma_start(out=rstd_row[0:1, c * 512:(c + 1) * 512],
                                      in_=ag_out[c:c + 1, :])

            # ---- weights / tables (prefetch while stats run); phase-B-only
            # tensors live in the attention-scoped pool so MLP gets the space
            attn_pool_cm = tc.tile_pool(name="attn", bufs=1)
            pattn = attn_pool_cm.__enter__()
            # full-width causal masks for the 4 diagonal offsets: for a k-tile
            # at columns [off, off+128) keep where q - k = f - p - off >= 0
            maskfull = []
            for oi in range(4):
                mf = pattn.tile([128, 512], bf16, name=f"mask{oi}", tag=f"mask{oi}")
                nc.gpsimd.memset(mf, 1.0)
                nc.gpsimd.affine_select(
                    out=mf, in_=mf, pattern=[[1, 512]],
                    compare_op=OP.is_ge, fill=0.0, base=-oi * 128,
                    channel_multiplier=-1)
                maskfull.append(mf)
            wq_sb = {}
            for m in range(3):
                w = pattn.tile([128, 8, 128], bf16, name=f"wqkv_{m}", tag=f"wqkv_{m}")
                nc.sync.dma_start(out=w, in_=wqkv_blk[m])
                wq_sb[m] = w
            # wout/b2 tiles allocated now, DMA'd during attention (phase C)
            wo_sb = {m: pconst.tile([128, 8, 128], bf16, name=f"wout_{m}",
                                    tag=f"wout_{m}") for m in range(8)}
            b2_sb = {m: pconst.tile([128, 1], f32, name=f"b2_{m}", tag=f"b2_{m}")
                     for m in range(8)}

            # =============================================================
            # Phase B: QKV projection + RoPE + V transpose (pipelined)
            # =============================================================
            qT_sb = pattn.tile([128, T], bf16)
            kT_sb = pattn.tile([128, T], bf16)
            v_all = pattn.tile([128, NT, 130], bf16)

            # ---- QKV mains for ALL chunks first: none of this needs the
            # AllGather'd rstd, so the PE stays busy through the collective.
            # Raw QKV is evicted to SBUF via the (otherwise idle) scalar engine.
            raws = {}
            with tc.tile_pool(name="psQKV", bufs=6, space="PSUM") as psQ:
                for ch in range(NCH):
                    xrt = pbig2.tile([128, 8, 512], bf16, tag="xTr")
                    nc.sync.dma_start(out=xrt, in_=xT_blk[ch])
                    rs = []
                    for m in range(3):
                        ps = psQ.tile([128, 512], f32, tag="qkv")
                        for kk in range(8):
                            nc.tensor.matmul(ps, wq_sb[m][:, kk, :], xrt[:, kk, :],
                                             start=(kk == 0), stop=(kk == 7))
                        r = pattn.tile([128, 512], bf16, name=f"raw_{ch}_{m}",
                                       tag=f"raw_{ch}_{m}")
                        nc.scalar.activation(out=r, in_=ps, func=AF.Copy)
                        rs.append(r)
                    raws[ch] = rs

            # out-proj weights/bias stream in while attention computes
            for m in range(8):
                nc.sync.dma_start(out=wo_sb[m][:, 0:4, :], in_=wout_blk[m, :, 0:4, :])
                nc.sync.dma_start(out=wo_sb[m][:, 4:8, :], in_=wout_blk[m, :, 4:8, :])
                nc.sync.dma_start(out=b2_sb[m], in_=b2_t[m])

            nc.vector.memset(v_all[:, :, 64:65], 1.0)
            nc.vector.memset(v_all[:, :, 129:130], 1.0)

            with tc.tile_pool(name="psVT", bufs=1, space="PSUM") as psVT, \
                 tc.tile_pool(name="psB", bufs=1, space="PSUM") as psB, \
                 tc.tile_pool(name="psSC", bufs=4, space="PSUM") as psSC, \
                 tc.tile_pool(name="psO", bufs=2, space="PSUM") as psO:

                def bcastfold(ch):
                    sl = slice(ch * 512, (ch + 1) * 512)
                    ps_b = psB.tile([128, 512], f32, tag="bc")
                    nc.tensor.matmul(ps_b, ones_row[0:1, 0:128], rstd_row[0:1, sl],
                                     start=True, stop=True)
                    rstd_sb = pwork.tile([128, 512], bf16, tag="rstd", bufs=2)
                    nc.scalar.activation(out=rstd_sb, in_=ps_b, func=AF.Copy)
                    # rope tables stream per chunk; rstd folds in place
                    tab_c = pwork.tile([128, 512], bf16, tag="tabc", bufs=2)
                    nc.sync.dma_start(out=tab_c, in_=tab[0, :, sl])
                    tab_s = pwork.tile([128, 512], bf16, tag="tabs", bufs=2)
                    nc.sync.dma_start(out=tab_s, in_=tab[1, :, sl])
                    nc.vector.tensor_mul(out=tab_c, in0=tab_c, in1=rstd_sb)
                    nc.vector.tensor_mul(out=tab_s, in0=tab_s, in1=rstd_sb)
                    return rstd_sb, tab_c, tab_s

                def evict(ch, fold):
                    rstd_sb, tab_c, tab_s = fold
                    sl = slice(ch * 512, (ch + 1) * 512)
                    for m in range(2):  # q, k: rope from the raw SBUF copy
                        raw = raws[ch][m]
                        dst = qT_sb if m == 0 else kT_sb
                        tsw = pwork.tile([128, 512], bf16, tag="ropesw", bufs=2)
                        for h in range(2):
                            for a2 in range(2):
                                nc.vector.tensor_copy(
                                    out=tsw[h * 64 + a2 * 32:h * 64 + a2 * 32 + 32, :],
                                    in_=raw[h * 64 + (1 - a2) * 32:h * 64 + (1 - a2) * 32 + 32, :])
                        t1 = pwork.tile([128, 512], bf16, tag="ropet1", bufs=2)
                        nc.vector.tensor_mul(out=t1, in0=raw, in1=tab_c)
                        t2 = pwork.tile([128, 512], bf16, tag="ropet2", bufs=2)
                        nc.vector.tensor_mul(out=t2, in0=tsw, in1=tab_s)
                        nc.vector.tensor_add(out=dst[:, sl], in0=t1, in1=t2)
                    # v: scale by rstd, transpose to [t, e] tiles
                    vt = pwork.tile([128, 512], bf16, tag="vtmp")
                    nc.vector.tensor_mul(out=vt, in0=raws[ch][2], in1=rstd_sb)
                    for j in range(4):
                        g = ch * 4 + j
                        pst = psVT.tile([128, 128], bf16, tag="vtr")
                        nc.tensor.transpose(out=pst, in_=vt[:, j * 128:(j + 1) * 128],
                                            identity=ident_bf)
                        nc.vector.tensor_copy(out=v_all[:, g, 0:64], in_=pst[:, 0:64])
                        nc.vector.tensor_copy(out=v_all[:, g, 65:129], in_=pst[:, 64:128])

                def attn_hb(h, b):
                    hsl = slice(h * 64, (h + 1) * 64)
                    for qc in range(4):
                        qsl = slice(b * 2048 + qc * 512, b * 2048 + (qc + 1) * 512)
                        nkt = 4 * (qc + 1)
                        ps_o = psO.tile([65, 512], f32, tag="o")
                        p_prev = None
                        for kt in range(nkt):
                            ksl = slice(b * 2048 + kt * 128, b * 2048 + (kt + 1) * 128)
                            ps_s = psSC.tile([128, 512], f32, tag="sc")
                            nc.tensor.matmul(ps_s, kT_sb[hsl, ksl], qT_sb[hsl, qsl],
                                             start=True, stop=True)
                            p_t = pwork.tile([128, 512], bf16, tag="p", bufs=6)
                            nc.scalar.activation(out=p_t, in_=ps_s, func=AF.Exp)
                            if kt >= 4 * qc:  # diagonal block: causal mask
                                nc.gpsimd.tensor_mul(out=p_t, in0=p_t,
                                                     in1=maskfull[kt - 4 * qc])
                            if p_prev is not None:
                                g, pp, first = p_prev
                                nc.tensor.matmul(
                                    ps_o, v_all[:, g, h * 65:(h + 1) * 65], pp,
                                    start=first, stop=False)
                            p_prev = (b * 16 + kt, p_t, kt == 0)
                        g, pp, first = p_prev
                        nc.tensor.matmul(ps_o, v_all[:, g, h * 65:(h + 1) * 65], pp,
                                         start=first, stop=True)
                        # ship UNNORMALIZED o + sums; divide after the AllToAll
                        o_u = pwork.tile([65, 512], bf16, tag="o_u", bufs=3)
                        nc.vector.tensor_copy(out=o_u, in_=ps_o)
                        nc.sync.dma_start(out=cc_in[h][b * 4 + qc], in_=o_u)

                for ch in range(4):
                    evict(ch, bcastfold(ch))
                attn_hb(0, 0)
                attn_hb(1, 0)
                for ch in range(4, NCH):
                    evict(ch, bcastfold(ch))
                attn_hb(0, 1)
                nc.gpsimd.collective_compute(
                    "AllToAll", OP.bypass, ins=[cc_in[0][:, :, :]],
                    outs=[cc_out[0][:, :, :]],
                    replica_groups=[list(range(N_CORES))])
                attn_hb(1, 1)

            nc.gpsimd.collective_compute(
                "AllToAll", OP.bypass, ins=[cc_in[1][:, :, :]],
                outs=[cc_out[1][:, :, :]],
                replica_groups=[list(range(N_CORES))])

            attn_pool_cm.__exit__(None, None, None)
            mlp_pool_cm = tc.tile_pool(name="mlp", bufs=1)
            pmlp = mlp_pool_cm.__enter__()

            # =============================================================
            # Phase D: out-proj (token-split), residual, LN2 stats
            # =============================================================
            pD_cm = tc.tile_pool(name="pD", bufs=1)
            pD = pD_cm.__enter__()
            o_own = [pD.tile([128, 512], bf16, name=f"oo_{kk}", tag=f"oo_{kk}")
                     for kk in range(8)]
            den_all = pD.tile([16, 512], bf16)
            # two wide DMAs pull all 16 denominator rows at once
            nc.sync.dma_start(out=den_all[0:8, :], in_=cc_out[0][:, 64, :])
            nc.sync.dma_start(out=den_all[8:16, :], in_=cc_out[1][:, 64, :])
            for kk in range(8):
                nc.sync.dma_start(out=o_own[kk][0:64, :], in_=cc_out[0][kk, 0:64, :])
                nc.sync.dma_start(out=o_own[kk][64:128, :], in_=cc_out[1][kk, 0:64, :])
            # ONE batched in-place reciprocal, then per-kk selector broadcast
            # (sel16 picks rows kk / kk+8) + in-place multiply normalizes o_own
            nc.vector.reciprocal(out=den_all, in_=den_all)
            with tc.tile_pool(name="psNB", bufs=2, space="PSUM") as psNB:
                for kk in range(8):
                    ps_nb = psNB.tile([128, 512], f32, tag="nb")
                    nc.tensor.matmul(ps_nb, sel16[:, kk * 128:(kk + 1) * 128],
                                     den_all, start=True, stop=True)
                    nc.vector.tensor_mul(out=o_own[kk], in0=o_own[kk], in1=ps_nb)

            xa = [pmlp.tile([128, 512], f32, name=f"xa_{m}", tag=f"xa_{m}") for m in range(8)]
            xab = [pmlp.tile([128, 512], bf16, name=f"xab_{m}", tag=f"xab_{m}") for m in range(8)]
            with tc.tile_pool(name="psOP", bufs=2, space="PSUM") as psOP, \
                 tc.tile_pool(name="psMU", bufs=1, space="PSUM") as psMU, \
                 tc.tile_pool(name="psSQ", bufs=1, space="PSUM") as psSQ, \
                 tc.tile_pool(name="psRB", bufs=1, space="PSUM") as psRB:
                ps_mu = psMU.tile([1, 512], f32)
                ps_sq = psSQ.tile([1, 512], f32)
                for m in range(8):
                    ps = psOP.tile([128, 512], f32, tag="op")
                    for kk in range(8):
                        nc.tensor.matmul(ps, wo_sb[m][:, kk, :], o_own[kk],
                                         start=(kk == 0), stop=(kk == 7))
                    nc.vector.tensor_add(out=xa[m], in0=ps, in1=xT_o[m])
                    nc.vector.tensor_copy(out=xab[m], in_=xa[m])
                    sq = pwork.tile([128, 512], bf16, tag="sq", bufs=2)
                    nc.vector.tensor_mul(out=sq, in0=xab[m], in1=xab[m])
                    nc.tensor.matmul(ps_mu, ones_col, xab[m],
                                     start=(m == 0), stop=(m == 7))
                    nc.tensor.matmul(ps_sq, ones_col, sq,
                                     start=(m == 0), stop=(m == 7))
                    # fold the mlp2 bias into the residual now (stats above
                    # already consumed the pre-bias value via xab/sq)
                    nc.scalar.activation(out=xa[m], in_=xa[m], func=AF.Identity,
                                         bias=b2_sb[m])

                # LN2 row stats: mu = sum/1024, var = sqsum/1024 - mu^2
                mu2 = pwork.tile([1, 512], f32, tag="mu2", bufs=1)
                nc.vector.tensor_scalar_mul(out=mu2, in0=ps_mu, scalar1=1.0 / D)
                mus_r = pwork.tile([1, 512], f32, tag="mus2_r", bufs=1)
                nc.vector.tensor_mul(out=mus_r, in0=mu2, in1=mu2)
                var_r = pwork.tile([1, 512], f32, tag="var2_r", bufs=1)
                nc.vector.scalar_tensor_tensor(
                    out=var_r, in0=ps_sq, scalar=1.0 / D, in1=mus_r,
                    op0=OP.mult, op1=OP.subtract)
                sd2 = pwork.tile([1, 512], f32, tag="sd2_r", bufs=1)
                nc.scalar.activation(out=sd2, in_=var_r, func=AF.Sqrt, bias=eps1)
                rstd2 = pwork.tile([1, 512], bf16, tag="rstd2_r", bufs=1)
                nc.vector.reciprocal(out=rstd2, in_=sd2)
                ps_rb = psRB.tile([128, 512], f32)
                nc.tensor.matmul(ps_rb, ones_row[0:1, 0:128], rstd2, start=True, stop=True)
                rstd2_sb = pmlp.tile([128, 512], bf16)
                nc.vector.tensor_copy(out=rstd2_sb, in_=ps_rb)
            pD_cm.__exit__(None, None, None)

            # =============================================================
            # Phase E: MLP (token-split, full weights)
            # =============================================================
            u_g = [pmlp.tile([128, 512], bf16, name=f"ug_{m}", tag=f"ug_{m}") for m in range(32)]
            with tc.tile_pool(name="psU", bufs=3, space="PSUM") as psU, \
                 tc.tile_pool(name="psDn", bufs=2, space="PSUM") as psDn:
                for m in range(32):
                    ps = psU.tile([128, 512], f32, tag="u")
                    w = pstream.tile([128, 8, 128], bf16, tag="w1_st", bufs=4)
                    nc.sync.dma_start(out=w[:, 0:4, :], in_=w1_blk[m, :, 0:4, :])
                    nc.sync.dma_start(out=w[:, 4:8, :], in_=w1_blk[m, :, 4:8, :])
                    for kk in range(8):
                        nc.tensor.matmul(ps, w[:, kk, :], xab[kk],
                                         start=(kk == 0), stop=(kk == 7))
                    upre = pwork.tile([128, 512], bf16, tag="upre", bufs=2)
                    nc.vector.tensor_mul(out=upre, in0=ps, in1=rstd2_sb)
                    b1 = pwork.tile([128, 1], f32, tag="b1_st")
                    nc.sync.dma_start(out=b1, in_=b1_t[m])
                    nc.scalar.activation(out=u_g[m], in_=upre,
                                         func=AF.Gelu_apprx_tanh, bias=b1)
                for m in range(8):
                    ps = psDn.tile([128, 512], f32, tag="dn")
                    w = pstream.tile([128, 32, 128], bf16, tag="w2_st", bufs=2)
                    for q4 in range(4):
                        nc.sync.dma_start(out=w[:, q4 * 8:(q4 + 1) * 8, :],
                                          in_=w2_blk[m, :, q4 * 8:(q4 + 1) * 8, :])
                    for kk in range(32):
                        nc.tensor.matmul(ps, w[:, kk, :], u_g[kk],
                                         start=(kk == 0), stop=(kk == 31))
                    ot = pwork.tile([128, 512], f32, tag="ot", bufs=2)
                    nc.vector.tensor_add(out=ot, in0=ps, in1=xa[m])
                    nc.sync.dma_start(out=out_d[m * 128:(m + 1) * 128, :], in_=ot)
            mlp_pool_cm.__exit__(None, None, None)

    _legalize_sync(nc)
    return nc


# ---------------------------------------------------------------------------
# Host-side prep + execution
# ---------------------------------------------------------------------------
_NC_CACHE = {}


def _get_nc():
    if "nc" not in _NC_CACHE:
        _NC_CACHE["nc"] = _build()
    return _NC_CACHE["nc"]


def _bf(a):
    return np.ascontiguousarray(a).astype(ml_dtypes.bfloat16)


def _f32(a):
    return np.ascontiguousarray(a, dtype=np.float32)


def _mean_fold(w):
    # W' = W - rowsum(W)/D : folds LN mean-centering into the matmul
    return w - w.sum(1, keepdims=True) / w.shape[1]


def _prep_inputs(x, rot_cos, rot_sin, ln1_w, w_qkv, w_out, ln2_w, w_mlp1,
                 b_mlp1, w_mlp2, b_mlp2):
    x = np.asarray(x, np.float32)
    X = x.reshape(T, D)

    xT = X.T  # (D, T)
    # (ch, p, kk, t): partition row p holds all kk-blocks contiguously
    xT_blk = _bf(xT.reshape(8, 128, NCH, 512).transpose(2, 1, 0, 3))

    # rope tables: (128 rows = 2 heads x [first32|last32]) x T tokens
    cos = np.asarray(rot_cos, np.float32)[0, :, 0, 0, :HD // 2]  # (S, 32)
    sin = np.asarray(rot_sin, np.float32)[0, :, 0, 0, :HD // 2]
    cT = np.concatenate([cos, cos], 1).T          # (64, S)
    sT = np.concatenate([-sin, sin], 1).T         # (64, S) sign-folded
    cT = np.tile(cT, (2, B))                      # (128, T)
    sT = np.tile(sT, (2, B))
    tab = _bf(np.stack([cT, sT]))

    wqkv_eff = np.asarray(w_qkv, np.float32) * np.asarray(ln1_w, np.float32)[None, :]
    w1_eff = np.asarray(w_mlp1, np.float32) * np.asarray(ln2_w, np.float32)[None, :]
    w1_eff = _mean_fold(w1_eff)
    w_out_f = np.asarray(w_out, np.float32)
    w2_f = np.asarray(w_mlp2, np.float32)

    woutT = w_out_f.T  # (d_in=head dims, e)
    wout_blk = _bf(woutT.reshape(8, 128, 8, 128).transpose(2, 1, 0, 3))  # [m, p, kk, e]
    w1T = w1_eff.T     # (D, 4D)
    w1_blk = _bf(w1T.reshape(8, 128, 32, 128).transpose(2, 1, 0, 3))
    w2T = w2_f.T       # (4D, D)
    w2_blk = _bf(w2T.reshape(32, 128, 8, 128).transpose(2, 1, 0, 3))
    b1_arr = _f32(np.asarray(b_mlp1, np.float32).reshape(32, 128, 1))
    b2_arr = _f32(np.asarray(b_mlp2, np.float32).reshape(8, 128, 1))
    sel16 = np.zeros((16, 8 * 128), np.float32)
    for kk in range(8):
        sel16[kk, kk * 128:kk * 128 + 64] = 1.0
        sel16[kk + 8, kk * 128 + 64:(kk + 1) * 128] = 1.0
    sel16 = _bf(sel16)

    in_maps = []
    for c in range(N_CORES):
        w_sl = np.concatenate(
            [wqkv_eff[0 * D + 2 * c * HD: 0 * D + 2 * (c + 1) * HD] * 0.125,
             wqkv_eff[1 * D + 2 * c * HD: 1 * D + 2 * (c + 1) * HD],
             wqkv_eff[2 * D + 2 * c * HD: 2 * D + 2 * (c + 1) * HD]], 0)  # (384, D)
        w_sl = _mean_fold(w_sl)
        wT_sl = w_sl.T  # (D, 384) -> [m, p, kk, e]
        wqkv_b = _bf(wT_sl.reshape(8, 128, 3, 128).transpose(2, 1, 0, 3))
        in_maps.append({
            "xT_blk": xT_blk,
            "xT_own": _f32(xT[:, c * TOK:(c + 1) * TOK]),
            "wqkv_blk": wqkv_b,
            "tab": tab,
            "wout_blk": wout_blk,
            "w1_blk": w1_blk,
            "b1_t": b1_arr,
            "w2_blk": w2_blk,
            "b2_t": b2_arr,
            "sel2_d": sel16,
        })
    return in_maps


def _assemble(results):
    outT = np.concatenate([results[c]["out"] for c in range(N_CORES)], axis=1)
    return np.ascontiguousarray(outT.T.astype(np.float32)).reshape(B, S, D)


def run_spmd(in_maps, **kwargs):
    nc = _get_nc()
    return run_bass_kernel_spmd(nc, in_maps, core_ids=list(range(N_CORES)), **kwargs)


def kernel(x, rot_cos, rot_sin, ln1_w, w_qkv, w_out, ln2_w, w_mlp1, b_mlp1,
           w_mlp2, b_mlp2):
    in_maps = _prep_inputs(x, rot_cos, rot_sin, ln1_w, w_qkv, w_out, ln2_w,
                           w_mlp1, b_mlp1, w_mlp2, b_mlp2)
    res = run_spmd(in_maps)
    return _assemble(res.results)
